# revision 18
# baseline (speedup 1.0000x reference)
"""BevPoolV2 Trainium2 kernel (8 NeuronCores, SPMD, no collectives).

v6: fixed windows.  Stream A uses 1024 tiles with static windows
[4t, 4t+16); a point in cell x may sit in any of 4 candidate tiles
(greedy assignment on host), the mask encodes its window column.  Tiles
of phase t%4 write disjoint row ranges of phase slab oa<phi> with plain
streaming DMA -- no scatter, no RMW, no output descriptor generation.
Stream B (feat rows >= 32768, int16 gather-index limit) uses 64 tiles
with windows [64t, 64t+64) -> slab ob.  The handful of points that
overflow their 4 candidate tiles go to a 256-slot spill stream with
chained dma_scatter_add calls -> slab oc.  Host sums the six slabs.
Depth extraction: host ships one-hot lane masks (sel); DVE does mult
(2x bf16) + binary-tree adds + an 8-wide reduce.  Gathers spread over
all 4 SWDGE queues (Q7 core pairs).
"""
import numpy as np

B, N, D, H, W = 2, 6, 120, 32, 88
C = 128
NCELLS = 32768
NCORES = 8
CELLS_PER_CORE = NCELLS // NCORES   # 4096
TILE_P = 128
STRIDE, WIN = 4, 16                 # A window geometry
T_A = 1024                          # A tiles
CHUNK = 64                          # tiles per A-chunk
NCHUNK = 16
HALF = 32
NI = CHUNK * TILE_P                 # 8192 idxs per chunk
B_STRIDE = 64                       # B window width (= stride)
NB_T = 64                           # B tiles
NS = 256                            # spill slots (2 tiles)
NRANK = 8                           # chained spill scatter calls
DUMMY = CELLS_PER_CORE              # trash row in oc
N_FEAT_ROWS = B * N * H * W         # 33792
N_DEPTH = B * N * D * H * W         # 4055040
N_DEP_BLK = N_DEPTH // 128          # 31680
A_LIM = 32768


def _pack16(ent):
    """entry i -> int16 storage [i%16, i//16], replicated to 128 partitions."""
    a = np.asarray(ent, np.int16).reshape(-1, 16).T
    return np.ascontiguousarray(np.tile(a, (8, 1)))


def _bf16(x):
    import ml_dtypes
    return np.ascontiguousarray(np.asarray(x).astype(ml_dtypes.bfloat16))


# ---------------------------------------------------------------- host prep
def _assign_fixed(rb, n_tiles, stride, win):
    """Greedy earliest-tile assignment of sorted cells to fixed windows
    [stride*t, stride*t+win).  Returns tile_id per point (-1 = spill)."""
    fill = np.zeros(n_tiles, np.int32)
    tile_id = np.full(len(rb), -1, np.int64)
    cells, starts, counts = np.unique(rb, return_index=True,
                                      return_counts=True)
    for x, s0, n in zip(cells, starts, counts):
        t_hi = int(x) // stride
        t_lo = max(0, t_hi - (win // stride - 1))
        left = int(n)
        for t in range(t_lo, min(t_hi, n_tiles - 1) + 1):
            take = min(left, 128 - fill[t])
            if take > 0:
                i0 = s0 + n - left
                tile_id[i0:i0 + take] = t
                fill[t] += take
                left -= take
            if left == 0:
                break
    return tile_id


def _stream_arrays(rb, rf, rd, tile_id, n_tiles, stride, win):
    """Pack per-point data into [n_tiles, 128] slot arrays + masks/sel."""
    keep = tile_id >= 0
    rbk, rfk, rdk, tk = rb[keep], rf[keep], rd[keep], tile_id[keep]
    order = np.argsort(tk, kind="stable")
    rbk, rfk, rdk, tk = rbk[order], rfk[order], rdk[order], tk[order]
    fill = np.bincount(tk, minlength=n_tiles)
    t_start = np.concatenate([[0], np.cumsum(fill)])
    slot = np.arange(len(tk)) - t_start[tk]
    rf_t = np.zeros((n_tiles, TILE_P), np.int64)
    rd_t = np.zeros((n_tiles, TILE_P), np.int64)
    msk = np.zeros((n_tiles, TILE_P, win), np.float32)
    sel = np.zeros((n_tiles, TILE_P, 128), np.float32)
    rf_t[tk, slot] = rfk
    rd_t[tk, slot] = rdk
    msk[tk, slot, rbk - stride * tk] = 1.0
    sel[tk, slot, rdk % 128] = 1.0
    return rf_t, rd_t, msk, sel


def _preprocess(ranks_depth, ranks_feat, ranks_bev):
    ranks_bev = np.asarray(ranks_bev)
    ranks_feat = np.asarray(ranks_feat).astype(np.int64)
    ranks_depth = np.asarray(ranks_depth).astype(np.int64)
    bounds = np.searchsorted(ranks_bev, np.arange(0, NCELLS + 1, CELLS_PER_CORE))
    cores = []
    for k in range(NCORES):
        lo, hi = int(bounds[k]), int(bounds[k + 1])
        rb = ranks_bev[lo:hi].astype(np.int64) - k * CELLS_PER_CORE
        rf = ranks_feat[lo:hi]
        rd = ranks_depth[lo:hi]
        isB = rf >= A_LIM

        # ---------------- stream A ----------------
        rbA, rfA, rdA = rb[~isB], rf[~isB], rd[~isB]
        tidA = _assign_fixed(rbA, T_A, STRIDE, WIN)
        rfA_t, rdA_t, mskA_f, selA_f = _stream_arrays(
            rbA, rfA, rdA, tidA, T_A, STRIDE, WIN)
        rfiA = np.empty((NCHUNK, TILE_P, NI // 16), np.int16)
        rdiA = np.empty_like(rfiA)
        mskA = np.empty((NCHUNK, TILE_P, CHUNK * WIN), np.float32)
        selA = np.empty((NCHUNK, TILE_P, CHUNK * 128), np.float32)
        for c in range(NCHUNK):
            t0 = c * CHUNK
            rfiA[c] = _pack16(rfA_t[t0:t0 + CHUNK].reshape(-1))
            rdiA[c] = _pack16((rdA_t[t0:t0 + CHUNK] // 128).reshape(-1))
            mskA[c] = mskA_f[t0:t0 + CHUNK].transpose(1, 0, 2).reshape(
                TILE_P, CHUNK * WIN)
            selA[c] = selA_f[t0:t0 + CHUNK].transpose(1, 0, 2).reshape(
                TILE_P, CHUNK * 128)

        # ---------------- stream B ----------------
        rbB, rfB, rdB = rb[isB], rf[isB], rd[isB]
        tidB = np.asarray(rbB) // B_STRIDE
        assert np.bincount(tidB, minlength=NB_T).max() <= TILE_P, k
        rfB_t, rdB_t, mskB_f, selB_f = _stream_arrays(
            rbB, rfB - A_LIM, rdB, tidB, NB_T, B_STRIDE, B_STRIDE)
        rfiB = _pack16(rfB_t.reshape(-1))
        rdiB = _pack16((rdB_t // 128).reshape(-1))
        mskB = mskB_f.transpose(1, 0, 2).reshape(TILE_P, NB_T * B_STRIDE)
        selB = selB_f.transpose(1, 0, 2).reshape(TILE_P, NB_T * 128)

        # ---------------- spill stream ----------------
        spill = tidA < 0
        rbS, rfS, rdS = rbA[spill], rfA[spill], rdA[spill]
        nS = len(rbS)
        assert nS <= NS, (k, nS)
        rfS_e = np.zeros(NS, np.int64)
        rdS_e = np.zeros(NS, np.int64)
        rfS_e[:nS] = rfS
        rdS_e[:nS] = rdS
        rfiS = _pack16(rfS_e)
        rdiS = _pack16(rdS_e // 128)
        # slot i -> partition i%128, block i//128 (gather row layout)
        selS = np.zeros((TILE_P, (NS // 128) * 128), np.float32)
        i_s = np.arange(nS)
        selS[i_s % 128, (i_s // 128) * 128 + (rdS % 128)] = 1.0
        # scatter entries: NRANK chained calls (dup cells split by rank)
        sidxS = np.full((NRANK, NS), DUMMY, np.int64)
        seen = {}
        for i in range(nS):
            cell = int(rbS[i])
            r = seen.get(cell, 0)
            assert r < NRANK, (k, cell)
            sidxS[r, i] = cell
            seen[cell] = r + 1
        sidxS_p = np.concatenate([_pack16(sidxS[r]) for r in range(NRANK)],
                                 axis=1)

        cores.append(dict(rfiA=rfiA, rdiA=rdiA, mskA=_bf16(mskA),
                          selA=_bf16(selA), rfiB=rfiB, rdiB=rdiB,
                          mskB=_bf16(mskB), selB=_bf16(selB),
                          rfiS=rfiS, rdiS=rdiS, selS=_bf16(selS),
                          sidxS=sidxS_p))
    return cores


# ---------------------------------------------------------------- program
_CACHED = {}


def _build_program():
    import concourse.bass as bass
    import concourse.bacc as bacc
    import concourse.tile as tile
    from concourse import mybir
    from concourse.tile import add_dep_helper

    nc = bacc.Bacc("TRN2", target_bir_lowering=False, debug=False,
                   num_swdge_queues=4)
    f32, bf16, i16 = mybir.dt.float32, mybir.dt.bfloat16, mybir.dt.int16
    feat_t = nc.dram_tensor("feat_tbl", [N_FEAT_ROWS, C // 2], f32,
                            kind="ExternalInput").ap()
    dep_t = nc.dram_tensor("dep_tbl", [N_DEP_BLK, 64], f32,
                           kind="ExternalInput").ap()
    rfiA_t = nc.dram_tensor("rfiA", [NCHUNK, TILE_P, NI // 16], i16,
                            kind="ExternalInput").ap()
    rdiA_t = nc.dram_tensor("rdiA", [NCHUNK, TILE_P, NI // 16], i16,
                            kind="ExternalInput").ap()
    mskA_t = nc.dram_tensor("mskA", [NCHUNK, TILE_P, CHUNK * WIN], bf16,
                            kind="ExternalInput").ap()
    selA_t = nc.dram_tensor("selA", [NCHUNK, TILE_P, CHUNK * 128], bf16,
                            kind="ExternalInput").ap()
    rfiB_t = nc.dram_tensor("rfiB", [TILE_P, NI // 16], i16,
                            kind="ExternalInput").ap()
    rdiB_t = nc.dram_tensor("rdiB", [TILE_P, NI // 16], i16,
                            kind="ExternalInput").ap()
    mskB_t = nc.dram_tensor("mskB", [TILE_P, NB_T * B_STRIDE], bf16,
                            kind="ExternalInput").ap()
    selB_t = nc.dram_tensor("selB", [TILE_P, NB_T * 128], bf16,
                            kind="ExternalInput").ap()
    rfiS_t = nc.dram_tensor("rfiS", [TILE_P, NS // 16], i16,
                            kind="ExternalInput").ap()
    rdiS_t = nc.dram_tensor("rdiS", [TILE_P, NS // 16], i16,
                            kind="ExternalInput").ap()
    selS_t = nc.dram_tensor("selS", [TILE_P, (NS // 128) * 128], bf16,
                            kind="ExternalInput").ap()
    sidxS_t = nc.dram_tensor("sidxS", [TILE_P, NRANK * NS // 16], i16,
                             kind="ExternalInput").ap()
    oa_t = [nc.dram_tensor(f"oa{p}", [4096 + WIN, C], f32,
                           kind="ExternalOutput").ap() for p in range(4)]
    ob_t = nc.dram_tensor("ob", [4096, C], f32, kind="ExternalOutput").ap()
    oc_t = nc.dram_tensor("oc", [CELLS_PER_CORE + 1, C], f32,
                          kind="ExternalOutput").ap()

    MUL, ADD = mybir.AluOpType.mult, mybir.AluOpType.add
    AXX = mybir.AxisListType.X

    with tile.TileContext(nc) as tc:
        with (
            tc.tile_pool(name="cst", bufs=1) as cst,
            tc.tile_pool(name="seq", bufs=3) as seq,
            tc.tile_pool(name="gp", bufs=2) as gp,
            tc.tile_pool(name="dp", bufs=3) as dp,
            tc.tile_pool(name="sp", bufs=2) as sp,
            tc.tile_pool(name="xp", bufs=3) as xp,
            tc.tile_pool(name="xb", bufs=2) as xb,
            tc.tile_pool(name="ps", bufs=6, space="PSUM") as ps,
            tc.tile_pool(name="psb", bufs=2, space="PSUM") as psb,
        ):
            # ---- B/S inputs ----
            rfiB_sb = cst.tile([TILE_P, NI // 16], i16)
            rdiB_sb = cst.tile([TILE_P, NI // 16], i16)
            mskB_sb = cst.tile([TILE_P, NB_T * B_STRIDE], bf16)
            rfiS_sb = cst.tile([TILE_P, NS // 16], i16)
            rdiS_sb = cst.tile([TILE_P, NS // 16], i16)
            selS_sb = cst.tile([TILE_P, (NS // 128) * 128], bf16)
            sidS_sb = cst.tile([TILE_P, NRANK * NS // 16], i16)
            nc.sync.dma_start(rfiB_sb[:], rfiB_t)
            nc.sync.dma_start(rdiB_sb[:], rdiB_t)
            nc.sync.dma_start(mskB_sb[:], mskB_t)
            nc.sync.dma_start(rfiS_sb[:], rfiS_t)
            nc.sync.dma_start(rdiS_sb[:], rdiS_t)
            nc.sync.dma_start(selS_sb[:], selS_t)
            nc.sync.dma_start(sidS_sb[:], sidxS_t)
            gB_sb = cst.tile([TILE_P, CHUNK * C // 2], f32)
            dbB_sb = cst.tile([TILE_P, CHUNK * 64], f32)
            gS_sb = cst.tile([TILE_P, (NS // 128) * C // 2], f32)
            dbS_sb = cst.tile([TILE_P, (NS // 128) * 64], f32)
            dB_sb = cst.tile([TILE_P, NB_T], bf16)
            adB_sb = cst.tile([TILE_P, NB_T * B_STRIDE], bf16)
            dS_sb = cst.tile([TILE_P, NS // 128], bf16)
            gsS_sb = cst.tile([TILE_P, (NS // 128) * C], f32)

            def extract(sel_sb, db2, d_out, njw):
                """d_out[p, j] = sum_k sel[p,j,k]*db[p,j,k]  (njw j's)."""
                nw = njw * 128
                nc.vector.tensor_tensor(
                    out=sel_sb[:, :nw], in0=sel_sb[:, :nw], in1=db2, op=MUL)
                p3 = sel_sb[:, :nw].rearrange("p (j e) -> p j e", e=128)
                w = 64
                while w >= 8:
                    nc.vector.tensor_tensor(
                        out=p3[:, :, :w], in0=p3[:, :, :w],
                        in1=p3[:, :, w:2 * w], op=ADD)
                    w //= 2
                with nc.allow_low_precision(reason="one-hot, exact"):
                    nc.vector.tensor_reduce(
                        out=d_out, in_=p3[:, :, :8], axis=AXX, op=ADD)

            # ---- stream A ----
            for c in range(NCHUNK):
                rfi_sb = seq.tile([TILE_P, NI // 16], i16, tag="rfi")
                rdi_sb = seq.tile([TILE_P, NI // 16], i16, tag="rdi")
                msk_sb = seq.tile([TILE_P, CHUNK * WIN], bf16, tag="msk")
                nc.sync.dma_start(rfi_sb[:], rfiA_t[c])
                nc.sync.dma_start(rdi_sb[:], rdiA_t[c])
                nc.sync.dma_start(msk_sb[:], mskA_t[c])
                sel_sbs = []
                for h in range(2):
                    s_sb = sp.tile([TILE_P, NI // 2], bf16, tag="sel")
                    nc.sync.dma_start(
                        s_sb[:],
                        selA_t[c][:, h * (NI // 2):(h + 1) * (NI // 2)])
                    sel_sbs.append(s_sb)

                g_sb = gp.tile([TILE_P, CHUNK * C // 2], f32, tag="g")
                db_sb = gp.tile([TILE_P, CHUNK * 64], f32, tag="db")
                g3f = g_sb[:].rearrange("p (j e) -> p j e", e=C // 2)
                db3f = db_sb[:].rearrange("p (j e) -> p j e", e=64)
                HN = NI // 2
                nc.gpsimd.dma_gather(g3f[:, :HALF, :], feat_t,
                                     rfi_sb[:, :HN // 16], HN, HN, C // 2,
                                     single_packet=False, queue_num=c % 4)
                nc.gpsimd.dma_gather(g3f[:, HALF:, :], feat_t,
                                     rfi_sb[:, HN // 16:], HN, HN, C // 2,
                                     single_packet=False,
                                     queue_num=(c + 1) % 4)
                nc.gpsimd.dma_gather(db3f[:, :HALF, :], dep_t,
                                     rdi_sb[:, :HN // 16], HN, HN, 64,
                                     single_packet=False,
                                     queue_num=(c + 2) % 4)
                nc.gpsimd.dma_gather(db3f[:, HALF:, :], dep_t,
                                     rdi_sb[:, HN // 16:], HN, HN, 64,
                                     single_packet=False,
                                     queue_num=(c + 3) % 4)
                if c == 6:
                    gB3f = gB_sb[:].rearrange("p (j e) -> p j e", e=C // 2)
                    dbB3f = dbB_sb[:].rearrange("p (j e) -> p j e", e=64)
                    nc.gpsimd.dma_gather(gB3f[:, :HALF, :],
                                         feat_t[A_LIM:, :],
                                         rfiB_sb[:, :HN // 16], HN, HN,
                                         C // 2, single_packet=False,
                                         queue_num=0)
                    nc.gpsimd.dma_gather(gB3f[:, HALF:, :],
                                         feat_t[A_LIM:, :],
                                         rfiB_sb[:, HN // 16:], HN, HN,
                                         C // 2, single_packet=False,
                                         queue_num=1)
                if c == 7:
                    nc.gpsimd.dma_gather(dbB3f[:, :HALF, :], dep_t,
                                         rdiB_sb[:, :HN // 16], HN, HN, 64,
                                         single_packet=False, queue_num=2)
                    nc.gpsimd.dma_gather(dbB3f[:, HALF:, :], dep_t,
                                         rdiB_sb[:, HN // 16:], HN, HN, 64,
                                         single_packet=False, queue_num=3)
                if c == 12:
                    gS3f = gS_sb[:].rearrange("p (j e) -> p j e", e=C // 2)
                    dbS3f = dbS_sb[:].rearrange("p (j e) -> p j e", e=64)
                    nc.gpsimd.dma_gather(gS3f, feat_t, rfiS_sb[:],
                                         NS, NS, C // 2,
                                         single_packet=False, queue_num=2)
                    nc.gpsimd.dma_gather(dbS3f, dep_t, rdiS_sb[:],
                                         NS, NS, 64, single_packet=False,
                                         queue_num=3)

                g3 = g_sb[:].bitcast(bf16).rearrange("p (j e) -> p j e", e=C)
                db2 = db_sb[:].bitcast(bf16)

                d_sb = dp.tile([TILE_P, CHUNK], bf16, tag="d")
                for h in range(2):
                    extract(sel_sbs[h],
                            db2[:, h * (NI // 2):(h + 1) * (NI // 2)],
                            d_sb[:, h * HALF:(h + 1) * HALF], HALF)

                ad_sb = dp.tile([TILE_P, CHUNK * WIN], bf16, tag="ad")
                ad3 = ad_sb[:].rearrange("p (j w) -> p j w", w=WIN)
                nc.vector.tensor_tensor(
                    out=ad3,
                    in0=msk_sb[:].rearrange("p (j w) -> p j w", w=WIN),
                    in1=d_sb[:].to_broadcast([TILE_P, CHUNK, WIN]), op=MUL)

                # matmuls by phase; tile j = phi + 4u covers output rows
                # [256c + 4phi + 16u, +16) of slab oa[phi]
                for phi in range(4):
                    tmp_sb = xp.tile([WIN, 16 * C], f32, tag="tmp")
                    for gq in range(4):
                        pt = ps.tile([WIN, 4 * C], f32, tag="pt",
                                     space="PSUM")
                        for m in range(4):
                            j = phi + 4 * (4 * gq + m)
                            nc.tensor.matmul(out=pt[:, C * m:C * (m + 1)],
                                             lhsT=ad3[:, j, :],
                                             rhs=g3[:, j, :],
                                             start=True, stop=True)
                        nc.scalar.copy(
                            tmp_sb[:, 4 * C * gq:4 * C * (gq + 1)], pt[:])
                    r0 = 256 * c + 4 * phi
                    dst = oa_t[phi][r0:r0 + 256, :].rearrange(
                        "(u w) e -> w u e", u=16)
                    nc.scalar.dma_start(
                        dst, tmp_sb[:].rearrange("w (u e) -> w u e", e=C))

                if c == 9:
                    # B extraction mid-stream
                    dbB2 = dbB_sb[:].bitcast(bf16)
                    for hb in range(2):
                        selB_sb = sp.tile([TILE_P, NI // 2], bf16,
                                          tag="sel")
                        nc.sync.dma_start(
                            selB_sb[:],
                            selB_t[:, hb * (NI // 2):(hb + 1) * (NI // 2)])
                        extract(selB_sb,
                                dbB2[:, hb * (NI // 2):(hb + 1) * (NI // 2)],
                                dB_sb[:, hb * (NB_T // 2):
                                      (hb + 1) * (NB_T // 2)], NB_T // 2)
                if c == 10:
                    gB3 = gB_sb[:].bitcast(bf16).rearrange(
                        "p (j e) -> p j e", e=C)
                    adB3 = adB_sb[:].rearrange("p (j w) -> p j w",
                                               w=B_STRIDE)
                    nc.vector.tensor_tensor(
                        out=adB3,
                        in0=mskB_sb[:].rearrange("p (j w) -> p j w",
                                                 w=B_STRIDE),
                        in1=dB_sb[:].to_broadcast([TILE_P, NB_T, B_STRIDE]),
                        op=MUL)
                if c in (11, 12, 13, 14):
                    # B matmuls: 16 tiles -> ob rows [1024b, 1024b+1024)
                    bblk = c - 11
                    gB3 = gB_sb[:].bitcast(bf16).rearrange(
                        "p (j e) -> p j e", e=C)
                    adB3 = adB_sb[:].rearrange("p (j w) -> p j w",
                                               w=B_STRIDE)
                    tmpB_sb = xb.tile([B_STRIDE, 16 * C], f32, tag="tmpB")
                    for gq in range(4):
                        ptB = psb.tile([B_STRIDE, 4 * C], f32, tag="ptB",
                                       space="PSUM")
                        for m in range(4):
                            j = 16 * bblk + 4 * gq + m
                            nc.tensor.matmul(out=ptB[:, C * m:C * (m + 1)],
                                             lhsT=adB3[:, j, :],
                                             rhs=gB3[:, j, :],
                                             start=True, stop=True)
                        nc.scalar.copy(
                            tmpB_sb[:, 4 * C * gq:4 * C * (gq + 1)], ptB[:])
                    dst = ob_t[1024 * bblk:1024 * (bblk + 1), :].rearrange(
                        "(u w) e -> w u e", u=16)
                    nc.scalar.dma_start(
                        dst, tmpB_sb[:].rearrange("w (u e) -> w u e", e=C))

                if c == 14:
                    # spill products
                    dbS2 = dbS_sb[:].bitcast(bf16)
                    extract(selS_sb, dbS2[:], dS_sb[:], NS // 128)
                    gS3 = gS_sb[:].bitcast(bf16).rearrange(
                        "p (j e) -> p j e", e=C)
                    gsS3 = gsS_sb[:].rearrange("p (j e) -> p j e", e=C)
                    nc.vector.tensor_tensor(
                        out=gsS3, in0=gS3,
                        in1=dS_sb[:].to_broadcast(
                            [TILE_P, NS // 128, C]), op=MUL)

            # spill scatters (chained: call r holds rank-r duplicate cells)
            gsS3 = gsS_sb[:].rearrange("p (j e) -> p j e", e=C)
            prev = None
            for r in range(NRANK):
                sc = nc.gpsimd.dma_scatter_add(
                    oc_t, gsS3,
                    sidS_sb[:, r * NS // 16:(r + 1) * NS // 16],
                    NS, NS, C, single_packet=False, queue_num=r % 4)
                if prev is not None:
                    add_dep_helper(sc.ins, prev.ins, reason="spill chain")
                prev = sc
    nc.compile()
    return nc


def _get_program():
    if "nc" not in _CACHED:
        _CACHED["nc"] = _build_program()
    return _CACHED["nc"]


# ---------------------------------------------------------------- entry
def kernel(depth, feat, ranks_depth, ranks_feat, ranks_bev,
           interval_starts=None, interval_lengths=None):
    from concourse import bass_utils

    depth = np.asarray(depth, dtype=np.float32)
    feat = np.asarray(feat, dtype=np.float32)
    feat_flat = _bf16(feat.transpose(0, 1, 3, 4, 2).reshape(-1, C)) \
        .view(np.float32)
    dep_blk = _bf16(depth.reshape(N_DEP_BLK, 128)).view(np.float32)

    cores = _preprocess(ranks_depth, ranks_feat, ranks_bev)
    in_maps = []
    for k in range(NCORES):
        cd = cores[k]
        in_maps.append({
            "feat_tbl": feat_flat, "dep_tbl": dep_blk,
            "rfiA": cd["rfiA"], "rdiA": cd["rdiA"], "mskA": cd["mskA"],
            "selA": cd["selA"], "rfiB": cd["rfiB"], "rdiB": cd["rdiB"],
            "mskB": cd["mskB"], "selB": cd["selB"], "rfiS": cd["rfiS"],
            "rdiS": cd["rdiS"], "selS": cd["selS"], "sidxS": cd["sidxS"],
        })

    nc = _get_program()
    res = bass_utils.run_bass_kernel_spmd(nc, in_maps,
                                          core_ids=list(range(NCORES)))
    _CACHED["last_results"] = res

    out_full = np.zeros((B, C, 1, 128, 128), np.float32)
    for k in range(NCORES):
        r = res.results[k]
        oc = sum(np.asarray(r[f"oa{p}"])[:CELLS_PER_CORE] for p in range(4))
        oc = oc + np.asarray(r["ob"]) + np.asarray(r["oc"])[:CELLS_PER_CORE]
        b, blk = k // 4, k % 4
        out_full[b, :, 0, 32 * blk:32 * (blk + 1), :] = \
            oc.T.reshape(C, 32, 128)
    return out_full


# revision 19
# speedup vs baseline: 1.0884x; 1.0884x over previous
"""BevPoolV2 Trainium2 kernel (8 NeuronCores, SPMD, no collectives).

v6: fixed windows.  Stream A uses 1024 tiles with static windows
[4t, 4t+16); a point in cell x may sit in any of 4 candidate tiles
(greedy assignment on host), the mask encodes its window column.  Tiles
of phase t%4 write disjoint row ranges of phase slab oa<phi> with plain
streaming DMA -- no scatter, no RMW, no output descriptor generation.
Stream B (feat rows >= 32768, int16 gather-index limit) uses 64 tiles
with windows [64t, 64t+64) -> slab ob.  The handful of points that
overflow their 4 candidate tiles go to a 256-slot spill stream with
chained dma_scatter_add calls -> slab oc.  Host sums the six slabs.
Depth extraction: host ships one-hot lane masks (sel); DVE does mult
(2x bf16) + binary-tree adds + an 8-wide reduce.  Gathers spread over
all 4 SWDGE queues (Q7 core pairs).
"""
import numpy as np

B, N, D, H, W = 2, 6, 120, 32, 88
C = 128
NCELLS = 32768
NCORES = 8
CELLS_PER_CORE = NCELLS // NCORES   # 4096
TILE_P = 128
STRIDE, WIN = 4, 16                 # A window geometry
T_A = 1024                          # A tiles
CHUNK = 64                          # tiles per A-chunk
NCHUNK = 16
HALF = 32
NI = CHUNK * TILE_P                 # 8192 idxs per chunk
B_STRIDE = 64                       # B window width (= stride)
NB_T = 64                           # B tiles
NS = 256                            # spill slots (2 tiles)
NRANK = 8                           # chained spill scatter calls
DUMMY = CELLS_PER_CORE              # trash row in oc
N_FEAT_ROWS = B * N * H * W         # 33792
N_DEPTH = B * N * D * H * W         # 4055040
N_DEP_BLK = N_DEPTH // 128          # 31680
A_LIM = 32768


def _pack16(ent):
    """entry i -> int16 storage [i%16, i//16], replicated to 128 partitions."""
    a = np.asarray(ent, np.int16).reshape(-1, 16).T
    return np.ascontiguousarray(np.tile(a, (8, 1)))


def _bf16(x):
    import ml_dtypes
    return np.ascontiguousarray(np.asarray(x).astype(ml_dtypes.bfloat16))


# ---------------------------------------------------------------- host prep
def _assign_fixed(rb, n_tiles, stride, win):
    """Greedy earliest-tile assignment of sorted cells to fixed windows
    [stride*t, stride*t+win).  Returns tile_id per point (-1 = spill)."""
    fill = np.zeros(n_tiles, np.int32)
    tile_id = np.full(len(rb), -1, np.int64)
    cells, starts, counts = np.unique(rb, return_index=True,
                                      return_counts=True)
    for x, s0, n in zip(cells, starts, counts):
        t_hi = int(x) // stride
        t_lo = max(0, t_hi - (win // stride - 1))
        left = int(n)
        for t in range(t_lo, min(t_hi, n_tiles - 1) + 1):
            take = min(left, 128 - fill[t])
            if take > 0:
                i0 = s0 + n - left
                tile_id[i0:i0 + take] = t
                fill[t] += take
                left -= take
            if left == 0:
                break
    return tile_id


def _stream_arrays(rb, rf, rd, tile_id, n_tiles, stride, win):
    """Pack per-point data into [n_tiles, 128] slot arrays + masks/sel."""
    keep = tile_id >= 0
    rbk, rfk, rdk, tk = rb[keep], rf[keep], rd[keep], tile_id[keep]
    order = np.argsort(tk, kind="stable")
    rbk, rfk, rdk, tk = rbk[order], rfk[order], rdk[order], tk[order]
    fill = np.bincount(tk, minlength=n_tiles)
    t_start = np.concatenate([[0], np.cumsum(fill)])
    slot = np.arange(len(tk)) - t_start[tk]
    rf_t = np.zeros((n_tiles, TILE_P), np.int64)
    rd_t = np.zeros((n_tiles, TILE_P), np.int64)
    msk = np.zeros((n_tiles, TILE_P, win), np.float32)
    sel = np.zeros((n_tiles, TILE_P, 128), np.float32)
    rf_t[tk, slot] = rfk
    rd_t[tk, slot] = rdk
    msk[tk, slot, rbk - stride * tk] = 1.0
    sel[tk, slot, rdk % 128] = 1.0
    return rf_t, rd_t, msk, sel


def _preprocess(ranks_depth, ranks_feat, ranks_bev):
    ranks_bev = np.asarray(ranks_bev)
    ranks_feat = np.asarray(ranks_feat).astype(np.int64)
    ranks_depth = np.asarray(ranks_depth).astype(np.int64)
    bounds = np.searchsorted(ranks_bev, np.arange(0, NCELLS + 1, CELLS_PER_CORE))
    cores = []
    for k in range(NCORES):
        lo, hi = int(bounds[k]), int(bounds[k + 1])
        rb = ranks_bev[lo:hi].astype(np.int64) - k * CELLS_PER_CORE
        rf = ranks_feat[lo:hi]
        rd = ranks_depth[lo:hi]
        isB = rf >= A_LIM

        # ---------------- stream A ----------------
        rbA, rfA, rdA = rb[~isB], rf[~isB], rd[~isB]
        tidA = _assign_fixed(rbA, T_A, STRIDE, WIN)
        rfA_t, rdA_t, mskA_f, selA_f = _stream_arrays(
            rbA, rfA, rdA, tidA, T_A, STRIDE, WIN)
        rfiA = np.empty((NCHUNK, TILE_P, NI // 16), np.int16)
        rdiA = np.empty_like(rfiA)
        mskA = np.empty((NCHUNK, TILE_P, CHUNK * WIN), np.float32)
        selA = np.empty((NCHUNK, TILE_P, CHUNK * 128), np.float32)
        for c in range(NCHUNK):
            t0 = c * CHUNK
            rfiA[c] = _pack16(rfA_t[t0:t0 + CHUNK].reshape(-1))
            rdiA[c] = _pack16((rdA_t[t0:t0 + CHUNK] // 128).reshape(-1))
            mskA[c] = mskA_f[t0:t0 + CHUNK].transpose(1, 0, 2).reshape(
                TILE_P, CHUNK * WIN)
            selA[c] = selA_f[t0:t0 + CHUNK].transpose(1, 0, 2).reshape(
                TILE_P, CHUNK * 128)

        # ---------------- stream B ----------------
        rbB, rfB, rdB = rb[isB], rf[isB], rd[isB]
        tidB = np.asarray(rbB) // B_STRIDE
        assert np.bincount(tidB, minlength=NB_T).max() <= TILE_P, k
        rfB_t, rdB_t, mskB_f, selB_f = _stream_arrays(
            rbB, rfB - A_LIM, rdB, tidB, NB_T, B_STRIDE, B_STRIDE)
        rfiB = _pack16(rfB_t.reshape(-1))
        rdiB = _pack16((rdB_t // 128).reshape(-1))
        mskB = mskB_f.transpose(1, 0, 2).reshape(TILE_P, NB_T * B_STRIDE)
        selB = selB_f.transpose(1, 0, 2).reshape(TILE_P, NB_T * 128)

        # ---------------- spill stream ----------------
        spill = tidA < 0
        rbS, rfS, rdS = rbA[spill], rfA[spill], rdA[spill]
        nS = len(rbS)
        assert nS <= NS, (k, nS)
        rfS_e = np.zeros(NS, np.int64)
        rdS_e = np.zeros(NS, np.int64)
        rfS_e[:nS] = rfS
        rdS_e[:nS] = rdS
        rfiS = _pack16(rfS_e)
        rdiS = _pack16(rdS_e // 128)
        # slot i -> partition i%128, block i//128 (gather row layout)
        selS = np.zeros((TILE_P, (NS // 128) * 128), np.float32)
        i_s = np.arange(nS)
        selS[i_s % 128, (i_s // 128) * 128 + (rdS % 128)] = 1.0
        # scatter entries: NRANK chained calls (dup cells split by rank)
        sidxS = np.full((NRANK, NS), DUMMY, np.int64)
        seen = {}
        for i in range(nS):
            cell = int(rbS[i])
            r = seen.get(cell, 0)
            assert r < NRANK, (k, cell)
            sidxS[r, i] = cell
            seen[cell] = r + 1
        sidxS_p = np.concatenate([_pack16(sidxS[r]) for r in range(NRANK)],
                                 axis=1)

        cores.append(dict(rfiA=rfiA, rdiA=rdiA, mskA=_bf16(mskA),
                          selA=_bf16(selA), rfiB=rfiB, rdiB=rdiB,
                          mskB=_bf16(mskB), selB=_bf16(selB),
                          rfiS=rfiS, rdiS=rdiS, selS=_bf16(selS),
                          sidxS=sidxS_p))
    return cores


# ---------------------------------------------------------------- program
_CACHED = {}


def _build_program():
    import concourse.bass as bass
    import concourse.bacc as bacc
    import concourse.tile as tile
    from concourse import mybir
    from concourse.tile import add_dep_helper

    nc = bacc.Bacc("TRN2", target_bir_lowering=False, debug=False,
                   num_swdge_queues=4)
    f32, bf16, i16 = mybir.dt.float32, mybir.dt.bfloat16, mybir.dt.int16
    feat_t = nc.dram_tensor("feat_tbl", [N_FEAT_ROWS, C // 2], f32,
                            kind="ExternalInput").ap()
    dep_t = nc.dram_tensor("dep_tbl", [N_DEP_BLK, 64], f32,
                           kind="ExternalInput").ap()
    rfiA_t = nc.dram_tensor("rfiA", [NCHUNK, TILE_P, NI // 16], i16,
                            kind="ExternalInput").ap()
    rdiA_t = nc.dram_tensor("rdiA", [NCHUNK, TILE_P, NI // 16], i16,
                            kind="ExternalInput").ap()
    mskA_t = nc.dram_tensor("mskA", [NCHUNK, TILE_P, CHUNK * WIN], bf16,
                            kind="ExternalInput").ap()
    selA_t = nc.dram_tensor("selA", [NCHUNK, TILE_P, CHUNK * 128], bf16,
                            kind="ExternalInput").ap()
    rfiB_t = nc.dram_tensor("rfiB", [TILE_P, NI // 16], i16,
                            kind="ExternalInput").ap()
    rdiB_t = nc.dram_tensor("rdiB", [TILE_P, NI // 16], i16,
                            kind="ExternalInput").ap()
    mskB_t = nc.dram_tensor("mskB", [TILE_P, NB_T * B_STRIDE], bf16,
                            kind="ExternalInput").ap()
    selB_t = nc.dram_tensor("selB", [TILE_P, NB_T * 128], bf16,
                            kind="ExternalInput").ap()
    rfiS_t = nc.dram_tensor("rfiS", [TILE_P, NS // 16], i16,
                            kind="ExternalInput").ap()
    rdiS_t = nc.dram_tensor("rdiS", [TILE_P, NS // 16], i16,
                            kind="ExternalInput").ap()
    selS_t = nc.dram_tensor("selS", [TILE_P, (NS // 128) * 128], bf16,
                            kind="ExternalInput").ap()
    sidxS_t = nc.dram_tensor("sidxS", [TILE_P, NRANK * NS // 16], i16,
                             kind="ExternalInput").ap()
    oa_t = [nc.dram_tensor(f"oa{p}", [4096 + WIN, C], f32,
                           kind="ExternalOutput").ap() for p in range(4)]
    ob_t = nc.dram_tensor("ob", [4096, C], f32, kind="ExternalOutput").ap()
    oc_t = nc.dram_tensor("oc", [CELLS_PER_CORE + 1, C], f32,
                          kind="ExternalOutput").ap()

    MUL, ADD = mybir.AluOpType.mult, mybir.AluOpType.add
    AXX = mybir.AxisListType.X

    with tile.TileContext(nc) as tc:
        with (
            tc.tile_pool(name="cst", bufs=1) as cst,
            tc.tile_pool(name="seq", bufs=3) as seq,
            tc.tile_pool(name="gp", bufs=2) as gp,
            tc.tile_pool(name="dp", bufs=3) as dp,
            tc.tile_pool(name="sp", bufs=4) as sp,
            tc.tile_pool(name="xp", bufs=2) as xp,
            tc.tile_pool(name="xb", bufs=1) as xb,
            tc.tile_pool(name="ps", bufs=6, space="PSUM") as ps,
            tc.tile_pool(name="psb", bufs=2, space="PSUM") as psb,
        ):
            # ---- B/S inputs ----
            rfiB_sb = cst.tile([TILE_P, NI // 16], i16)
            rdiB_sb = cst.tile([TILE_P, NI // 16], i16)
            mskB_sb = cst.tile([TILE_P, NB_T * B_STRIDE], bf16)
            rfiS_sb = cst.tile([TILE_P, NS // 16], i16)
            rdiS_sb = cst.tile([TILE_P, NS // 16], i16)
            selS_sb = cst.tile([TILE_P, (NS // 128) * 128], bf16)
            sidS_sb = cst.tile([TILE_P, NRANK * NS // 16], i16)
            nc.sync.dma_start(rfiB_sb[:], rfiB_t)
            nc.sync.dma_start(rdiB_sb[:], rdiB_t)
            nc.sync.dma_start(mskB_sb[:], mskB_t)
            nc.sync.dma_start(rfiS_sb[:], rfiS_t)
            nc.sync.dma_start(rdiS_sb[:], rdiS_t)
            nc.sync.dma_start(selS_sb[:], selS_t)
            nc.sync.dma_start(sidS_sb[:], sidxS_t)
            gB_sb = cst.tile([TILE_P, CHUNK * C // 2], f32)
            dbB_sb = cst.tile([TILE_P, CHUNK * 64], f32)
            gS_sb = cst.tile([TILE_P, (NS // 128) * C // 2], f32)
            dbS_sb = cst.tile([TILE_P, (NS // 128) * 64], f32)
            dB_sb = cst.tile([TILE_P, NB_T], bf16)
            adB_sb = cst.tile([TILE_P, NB_T * B_STRIDE], bf16)
            dS_sb = cst.tile([TILE_P, NS // 128], bf16)
            gsS_sb = cst.tile([TILE_P, (NS // 128) * C], f32)

            def extract(sel_sb, db2, d_out, njw):
                """d_out[p, j] = sum_k sel[p,j,k]*db[p,j,k]  (njw j's)."""
                nw = njw * 128
                nc.vector.tensor_tensor(
                    out=sel_sb[:, :nw], in0=sel_sb[:, :nw], in1=db2, op=MUL)
                p3 = sel_sb[:, :nw].rearrange("p (j e) -> p j e", e=128)
                w = 64
                while w >= 8:
                    nc.vector.tensor_tensor(
                        out=p3[:, :, :w], in0=p3[:, :, :w],
                        in1=p3[:, :, w:2 * w], op=ADD)
                    w //= 2
                with nc.allow_low_precision(reason="one-hot, exact"):
                    nc.vector.tensor_reduce(
                        out=d_out, in_=p3[:, :, :8], axis=AXX, op=ADD)

            # ---- stream A ----
            for c in range(NCHUNK):
                rfi_sb = seq.tile([TILE_P, NI // 16], i16, tag="rfi")
                rdi_sb = seq.tile([TILE_P, NI // 16], i16, tag="rdi")
                msk_sb = seq.tile([TILE_P, CHUNK * WIN], bf16, tag="msk")
                nc.sync.dma_start(rfi_sb[:], rfiA_t[c])
                nc.sync.dma_start(rdi_sb[:], rdiA_t[c])
                nc.sync.dma_start(msk_sb[:], mskA_t[c])
                sel_sbs = []
                for h in range(2):
                    s_sb = sp.tile([TILE_P, NI // 2], bf16, tag="sel")
                    nc.sync.dma_start(
                        s_sb[:],
                        selA_t[c][:, h * (NI // 2):(h + 1) * (NI // 2)])
                    sel_sbs.append(s_sb)

                g_sb = gp.tile([TILE_P, CHUNK * C // 2], f32, tag="g")
                db_sb = gp.tile([TILE_P, CHUNK * 64], f32, tag="db")
                g3f = g_sb[:].rearrange("p (j e) -> p j e", e=C // 2)
                db3f = db_sb[:].rearrange("p (j e) -> p j e", e=64)
                HN = NI // 2
                nc.gpsimd.dma_gather(g3f[:, :HALF, :], feat_t,
                                     rfi_sb[:, :HN // 16], HN, HN, C // 2,
                                     single_packet=False, queue_num=c % 4)
                nc.gpsimd.dma_gather(g3f[:, HALF:, :], feat_t,
                                     rfi_sb[:, HN // 16:], HN, HN, C // 2,
                                     single_packet=False,
                                     queue_num=(c + 1) % 4)
                nc.gpsimd.dma_gather(db3f[:, :HALF, :], dep_t,
                                     rdi_sb[:, :HN // 16], HN, HN, 64,
                                     single_packet=False,
                                     queue_num=(c + 2) % 4)
                nc.gpsimd.dma_gather(db3f[:, HALF:, :], dep_t,
                                     rdi_sb[:, HN // 16:], HN, HN, 64,
                                     single_packet=False,
                                     queue_num=(c + 3) % 4)
                if c == 6:
                    gB3f = gB_sb[:].rearrange("p (j e) -> p j e", e=C // 2)
                    dbB3f = dbB_sb[:].rearrange("p (j e) -> p j e", e=64)
                    nc.gpsimd.dma_gather(gB3f[:, :HALF, :],
                                         feat_t[A_LIM:, :],
                                         rfiB_sb[:, :HN // 16], HN, HN,
                                         C // 2, single_packet=False,
                                         queue_num=0)
                    nc.gpsimd.dma_gather(gB3f[:, HALF:, :],
                                         feat_t[A_LIM:, :],
                                         rfiB_sb[:, HN // 16:], HN, HN,
                                         C // 2, single_packet=False,
                                         queue_num=1)
                if c == 7:
                    nc.gpsimd.dma_gather(dbB3f[:, :HALF, :], dep_t,
                                         rdiB_sb[:, :HN // 16], HN, HN, 64,
                                         single_packet=False, queue_num=2)
                    nc.gpsimd.dma_gather(dbB3f[:, HALF:, :], dep_t,
                                         rdiB_sb[:, HN // 16:], HN, HN, 64,
                                         single_packet=False, queue_num=3)
                if c == 12:
                    gS3f = gS_sb[:].rearrange("p (j e) -> p j e", e=C // 2)
                    dbS3f = dbS_sb[:].rearrange("p (j e) -> p j e", e=64)
                    nc.gpsimd.dma_gather(gS3f, feat_t, rfiS_sb[:],
                                         NS, NS, C // 2,
                                         single_packet=False, queue_num=2)
                    nc.gpsimd.dma_gather(dbS3f, dep_t, rdiS_sb[:],
                                         NS, NS, 64, single_packet=False,
                                         queue_num=3)

                g3 = g_sb[:].bitcast(bf16).rearrange("p (j e) -> p j e", e=C)
                db2 = db_sb[:].bitcast(bf16)

                d_sb = dp.tile([TILE_P, CHUNK], bf16, tag="d")
                for h in range(2):
                    extract(sel_sbs[h],
                            db2[:, h * (NI // 2):(h + 1) * (NI // 2)],
                            d_sb[:, h * HALF:(h + 1) * HALF], HALF)

                ad_sb = dp.tile([TILE_P, CHUNK * WIN], bf16, tag="ad")
                ad3 = ad_sb[:].rearrange("p (j w) -> p j w", w=WIN)
                nc.vector.tensor_tensor(
                    out=ad3,
                    in0=msk_sb[:].rearrange("p (j w) -> p j w", w=WIN),
                    in1=d_sb[:].to_broadcast([TILE_P, CHUNK, WIN]), op=MUL)

                # matmuls by phase; tile j = phi + 4u covers output rows
                # [256c + 4phi + 16u, +16) of slab oa[phi]
                for phi in range(4):
                    tmp_sb = xp.tile([WIN, 16 * C], f32, tag="tmp")
                    for gq in range(4):
                        pt = ps.tile([WIN, 4 * C], f32, tag="pt",
                                     space="PSUM")
                        for m in range(4):
                            j = phi + 4 * (4 * gq + m)
                            nc.tensor.matmul(out=pt[:, C * m:C * (m + 1)],
                                             lhsT=ad3[:, j, :],
                                             rhs=g3[:, j, :],
                                             start=True, stop=True)
                        nc.scalar.copy(
                            tmp_sb[:, 4 * C * gq:4 * C * (gq + 1)], pt[:])
                    r0 = 256 * c + 4 * phi
                    dst = oa_t[phi][r0:r0 + 256, :].rearrange(
                        "(u w) e -> w u e", u=16)
                    nc.scalar.dma_start(
                        dst, tmp_sb[:].rearrange("w (u e) -> w u e", e=C))

                if c == 9:
                    # B extraction mid-stream
                    dbB2 = dbB_sb[:].bitcast(bf16)
                    for hb in range(2):
                        selB_sb = sp.tile([TILE_P, NI // 2], bf16,
                                          tag="sel")
                        nc.sync.dma_start(
                            selB_sb[:],
                            selB_t[:, hb * (NI // 2):(hb + 1) * (NI // 2)])
                        extract(selB_sb,
                                dbB2[:, hb * (NI // 2):(hb + 1) * (NI // 2)],
                                dB_sb[:, hb * (NB_T // 2):
                                      (hb + 1) * (NB_T // 2)], NB_T // 2)
                if c == 10:
                    gB3 = gB_sb[:].bitcast(bf16).rearrange(
                        "p (j e) -> p j e", e=C)
                    adB3 = adB_sb[:].rearrange("p (j w) -> p j w",
                                               w=B_STRIDE)
                    nc.vector.tensor_tensor(
                        out=adB3,
                        in0=mskB_sb[:].rearrange("p (j w) -> p j w",
                                                 w=B_STRIDE),
                        in1=dB_sb[:].to_broadcast([TILE_P, NB_T, B_STRIDE]),
                        op=MUL)
                if c in (11, 12, 13, 14):
                    # B matmuls: 16 tiles -> ob rows [1024b, 1024b+1024)
                    bblk = c - 11
                    gB3 = gB_sb[:].bitcast(bf16).rearrange(
                        "p (j e) -> p j e", e=C)
                    adB3 = adB_sb[:].rearrange("p (j w) -> p j w",
                                               w=B_STRIDE)
                    tmpB_sb = xb.tile([B_STRIDE, 16 * C], f32, tag="tmpB")
                    for gq in range(4):
                        ptB = psb.tile([B_STRIDE, 4 * C], f32, tag="ptB",
                                       space="PSUM")
                        for m in range(4):
                            j = 16 * bblk + 4 * gq + m
                            nc.tensor.matmul(out=ptB[:, C * m:C * (m + 1)],
                                             lhsT=adB3[:, j, :],
                                             rhs=gB3[:, j, :],
                                             start=True, stop=True)
                        nc.scalar.copy(
                            tmpB_sb[:, 4 * C * gq:4 * C * (gq + 1)], ptB[:])
                    dst = ob_t[1024 * bblk:1024 * (bblk + 1), :].rearrange(
                        "(u w) e -> w u e", u=16)
                    nc.scalar.dma_start(
                        dst, tmpB_sb[:].rearrange("w (u e) -> w u e", e=C))

                if c == 14:
                    # spill products
                    dbS2 = dbS_sb[:].bitcast(bf16)
                    extract(selS_sb, dbS2[:], dS_sb[:], NS // 128)
                    gS3 = gS_sb[:].bitcast(bf16).rearrange(
                        "p (j e) -> p j e", e=C)
                    gsS3 = gsS_sb[:].rearrange("p (j e) -> p j e", e=C)
                    nc.vector.tensor_tensor(
                        out=gsS3, in0=gS3,
                        in1=dS_sb[:].to_broadcast(
                            [TILE_P, NS // 128, C]), op=MUL)

            # spill scatters (chained: call r holds rank-r duplicate cells)
            gsS3 = gsS_sb[:].rearrange("p (j e) -> p j e", e=C)
            prev = None
            for r in range(NRANK):
                sc = nc.gpsimd.dma_scatter_add(
                    oc_t, gsS3,
                    sidS_sb[:, r * NS // 16:(r + 1) * NS // 16],
                    NS, NS, C, single_packet=False, queue_num=r % 4)
                if prev is not None:
                    add_dep_helper(sc.ins, prev.ins, reason="spill chain")
                prev = sc
    nc.compile()
    return nc


def _get_program():
    if "nc" not in _CACHED:
        _CACHED["nc"] = _build_program()
    return _CACHED["nc"]


# ---------------------------------------------------------------- entry
def kernel(depth, feat, ranks_depth, ranks_feat, ranks_bev,
           interval_starts=None, interval_lengths=None):
    from concourse import bass_utils

    depth = np.asarray(depth, dtype=np.float32)
    feat = np.asarray(feat, dtype=np.float32)
    feat_flat = _bf16(feat.transpose(0, 1, 3, 4, 2).reshape(-1, C)) \
        .view(np.float32)
    dep_blk = _bf16(depth.reshape(N_DEP_BLK, 128)).view(np.float32)

    cores = _preprocess(ranks_depth, ranks_feat, ranks_bev)
    in_maps = []
    for k in range(NCORES):
        cd = cores[k]
        in_maps.append({
            "feat_tbl": feat_flat, "dep_tbl": dep_blk,
            "rfiA": cd["rfiA"], "rdiA": cd["rdiA"], "mskA": cd["mskA"],
            "selA": cd["selA"], "rfiB": cd["rfiB"], "rdiB": cd["rdiB"],
            "mskB": cd["mskB"], "selB": cd["selB"], "rfiS": cd["rfiS"],
            "rdiS": cd["rdiS"], "selS": cd["selS"], "sidxS": cd["sidxS"],
        })

    nc = _get_program()
    res = bass_utils.run_bass_kernel_spmd(nc, in_maps,
                                          core_ids=list(range(NCORES)))
    _CACHED["last_results"] = res

    out_full = np.zeros((B, C, 1, 128, 128), np.float32)
    for k in range(NCORES):
        r = res.results[k]
        oc = sum(np.asarray(r[f"oa{p}"])[:CELLS_PER_CORE] for p in range(4))
        oc = oc + np.asarray(r["ob"]) + np.asarray(r["oc"])[:CELLS_PER_CORE]
        b, blk = k // 4, k % 4
        out_full[b, :, 0, 32 * blk:32 * (blk + 1), :] = \
            oc.T.reshape(C, 32, 128)
    return out_full


# revision 24
# speedup vs baseline: 1.1688x; 1.0739x over previous
"""BevPoolV2 Trainium2 kernel (8 NeuronCores, SPMD, no collectives).

v6: fixed windows.  Stream A uses 1024 tiles with static windows
[4t, 4t+16); a point in cell x may sit in any of 4 candidate tiles
(greedy assignment on host), the mask encodes its window column.  Tiles
of phase t%4 write disjoint row ranges of phase slab oa<phi> with plain
streaming DMA -- no scatter, no RMW, no output descriptor generation.
Stream B (feat rows >= 32768, int16 gather-index limit) uses 64 tiles
with windows [64t, 64t+64) -> slab ob.  The handful of points that
overflow their 4 candidate tiles go to a 256-slot spill stream with
chained dma_scatter_add calls -> slab oc.  Host sums the six slabs.
Depth extraction: host ships one-hot lane masks (sel); DVE does mult
(2x bf16) + binary-tree adds + an 8-wide reduce.  Gathers spread over
all 4 SWDGE queues (Q7 core pairs).
"""
import numpy as np

B, N, D, H, W = 2, 6, 120, 32, 88
C = 128
NCELLS = 32768
NCORES = 8
CELLS_PER_CORE = NCELLS // NCORES   # 4096
TILE_P = 128
STRIDE, WIN = 4, 16                 # A window geometry
T_A = 1024                          # A tiles
CHUNK = 64                          # tiles per A-chunk
NCHUNK = 16
HALF = 32
NI = CHUNK * TILE_P                 # 8192 idxs per chunk
B_STRIDE = 128                      # B window width (= stride)
NB_T = 32                           # B tiles
NS = 256                            # spill slots (2 tiles)
NRANK = 8                           # chained spill scatter calls
DUMMY = CELLS_PER_CORE              # trash row in oc
N_FEAT_ROWS = B * N * H * W         # 33792
N_DEPTH = B * N * D * H * W         # 4055040
N_DEP_BLK = N_DEPTH // 128          # 31680
A_LIM = 32768


def _pack16(ent):
    """entry i -> int16 storage [i%16, i//16], replicated to 128 partitions."""
    a = np.asarray(ent, np.int16).reshape(-1, 16).T
    return np.ascontiguousarray(np.tile(a, (8, 1)))


def _bf16(x):
    import ml_dtypes
    return np.ascontiguousarray(np.asarray(x).astype(ml_dtypes.bfloat16))


# ---------------------------------------------------------------- host prep
def _assign_fixed(rb, n_tiles, stride, win):
    """Greedy earliest-tile assignment of sorted cells to fixed windows
    [stride*t, stride*t+win).  Returns tile_id per point (-1 = spill)."""
    fill = np.zeros(n_tiles, np.int32)
    tile_id = np.full(len(rb), -1, np.int64)
    cells, starts, counts = np.unique(rb, return_index=True,
                                      return_counts=True)
    for x, s0, n in zip(cells, starts, counts):
        t_hi = int(x) // stride
        t_lo = max(0, t_hi - (win // stride - 1))
        left = int(n)
        for t in range(t_lo, min(t_hi, n_tiles - 1) + 1):
            take = min(left, 128 - fill[t])
            if take > 0:
                i0 = s0 + n - left
                tile_id[i0:i0 + take] = t
                fill[t] += take
                left -= take
            if left == 0:
                break
    return tile_id


def _stream_arrays(rb, rf, rd, tile_id, n_tiles, stride, win):
    """Pack per-point data into [n_tiles, 128] slot arrays + masks/sel."""
    keep = tile_id >= 0
    rbk, rfk, rdk, tk = rb[keep], rf[keep], rd[keep], tile_id[keep]
    order = np.argsort(tk, kind="stable")
    rbk, rfk, rdk, tk = rbk[order], rfk[order], rdk[order], tk[order]
    fill = np.bincount(tk, minlength=n_tiles)
    t_start = np.concatenate([[0], np.cumsum(fill)])
    slot = np.arange(len(tk)) - t_start[tk]
    rf_t = np.zeros((n_tiles, TILE_P), np.int64)
    rd_t = np.zeros((n_tiles, TILE_P), np.int64)
    msk = np.zeros((n_tiles, TILE_P, win), np.float32)
    sel = np.zeros((n_tiles, TILE_P, 128), np.float32)
    rf_t[tk, slot] = rfk
    rd_t[tk, slot] = rdk
    msk[tk, slot, rbk - stride * tk] = 1.0
    sel[tk, slot, rdk % 128] = 1.0
    return rf_t, rd_t, msk, sel


def _preprocess(ranks_depth, ranks_feat, ranks_bev):
    ranks_bev = np.asarray(ranks_bev)
    ranks_feat = np.asarray(ranks_feat).astype(np.int64)
    ranks_depth = np.asarray(ranks_depth).astype(np.int64)
    bounds = np.searchsorted(ranks_bev, np.arange(0, NCELLS + 1, CELLS_PER_CORE))
    cores = []
    for k in range(NCORES):
        lo, hi = int(bounds[k]), int(bounds[k + 1])
        rb = ranks_bev[lo:hi].astype(np.int64) - k * CELLS_PER_CORE
        rf = ranks_feat[lo:hi]
        rd = ranks_depth[lo:hi]
        isB = rf >= A_LIM

        # ---------------- stream A ----------------
        rbA, rfA, rdA = rb[~isB], rf[~isB], rd[~isB]
        tidA = _assign_fixed(rbA, T_A, STRIDE, WIN)
        rfA_t, rdA_t, mskA_f, selA_f = _stream_arrays(
            rbA, rfA, rdA, tidA, T_A, STRIDE, WIN)
        rfiA = np.empty((NCHUNK, TILE_P, NI // 16), np.int16)
        rdiA = np.empty_like(rfiA)
        mskA = np.empty((NCHUNK, TILE_P, CHUNK * WIN), np.float32)
        selA = np.empty((NCHUNK, TILE_P, CHUNK * 128), np.float32)
        for c in range(NCHUNK):
            t0 = c * CHUNK
            rfiA[c] = _pack16(rfA_t[t0:t0 + CHUNK].reshape(-1))
            rdiA[c] = _pack16((rdA_t[t0:t0 + CHUNK] // 128).reshape(-1))
            mskA[c] = mskA_f[t0:t0 + CHUNK].transpose(1, 0, 2).reshape(
                TILE_P, CHUNK * WIN)
            selA[c] = selA_f[t0:t0 + CHUNK].transpose(1, 0, 2).reshape(
                TILE_P, CHUNK * 128)

        # ---------------- stream B ----------------
        rbB, rfB, rdB = rb[isB], rf[isB], rd[isB]
        tidB = _assign_fixed(rbB, NB_T, B_STRIDE, B_STRIDE)
        rfB_t, rdB_t, mskB_f, selB_f = _stream_arrays(
            rbB, rfB - A_LIM, rdB, tidB, NB_T, B_STRIDE, B_STRIDE)
        rfiB = _pack16(rfB_t.reshape(-1))
        rdiB = _pack16((rdB_t // 128).reshape(-1))
        mskB = mskB_f.transpose(1, 0, 2).reshape(TILE_P, NB_T * B_STRIDE)
        selB = selB_f.transpose(1, 0, 2).reshape(TILE_P, NB_T * 128)

        # ---------------- spill stream ----------------
        # tile 0: A spills (full feat table); tile 1: B spills (B table)
        spA, spB = tidA < 0, tidB < 0
        nSA, nSB = int(spA.sum()), int(spB.sum())
        assert nSA <= 128 and nSB <= 128, (k, nSA, nSB)
        rbS = np.concatenate([rbA[spA], np.full(128 - nSA, -1),
                              rbB[spB], np.full(128 - nSB, -1)])
        rfS0 = np.zeros(128, np.int64)
        rfS0[:nSA] = rfA[spA]
        rfS1 = np.zeros(128, np.int64)
        rfS1[:nSB] = rfB[spB] - A_LIM
        rdS_e = np.zeros(NS, np.int64)
        rdS_e[:nSA] = rdA[spA]
        rdS_e[128:128 + nSB] = rdB[spB]
        rfiS0 = _pack16(rfS0)
        rfiS1 = _pack16(rfS1)
        rdiS = _pack16(rdS_e // 128)
        # slot i -> partition i%128, block i//128 (gather row layout)
        selS = np.zeros((TILE_P, (NS // 128) * 128), np.float32)
        live = rbS >= 0
        i_s = np.arange(NS)[live]
        selS[i_s % 128, (i_s // 128) * 128 + (rdS_e[live] % 128)] = 1.0
        # scatter entries: NRANK chained calls (dup cells split by rank);
        # dead entries go to per-entry trash rows so their RMWs parallelize
        sidxS = np.tile(DUMMY + np.arange(NS), (NRANK, 1))
        seen = {}
        for i in np.nonzero(live)[0]:
            cell = int(rbS[i])
            r = seen.get(cell, 0)
            assert r < NRANK, (k, cell)
            sidxS[r, i] = cell
            seen[cell] = r + 1
        sidxS_p = np.concatenate([_pack16(sidxS[r]) for r in range(NRANK)],
                                 axis=1)

        cores.append(dict(rfiA=rfiA, rdiA=rdiA, mskA=_bf16(mskA),
                          selA=_bf16(selA), rfiB=rfiB, rdiB=rdiB,
                          mskB=_bf16(mskB), selB=_bf16(selB),
                          rfiS0=rfiS0, rfiS1=rfiS1, rdiS=rdiS,
                          selS=_bf16(selS), sidxS=sidxS_p))
    return cores


# ---------------------------------------------------------------- program
_CACHED = {}


def _build_program():
    import concourse.bass as bass
    import concourse.bacc as bacc
    import concourse.tile as tile
    from concourse import mybir
    from concourse.tile import add_dep_helper

    nc = bacc.Bacc("TRN2", target_bir_lowering=False, debug=False,
                   num_swdge_queues=4)
    f32, bf16, i16 = mybir.dt.float32, mybir.dt.bfloat16, mybir.dt.int16
    feat_t = nc.dram_tensor("feat_tbl", [N_FEAT_ROWS, C // 2], f32,
                            kind="ExternalInput").ap()
    dep_t = nc.dram_tensor("dep_tbl", [N_DEP_BLK, 64], f32,
                           kind="ExternalInput").ap()
    rfiA_t = nc.dram_tensor("rfiA", [NCHUNK, TILE_P, NI // 16], i16,
                            kind="ExternalInput").ap()
    rdiA_t = nc.dram_tensor("rdiA", [NCHUNK, TILE_P, NI // 16], i16,
                            kind="ExternalInput").ap()
    mskA_t = nc.dram_tensor("mskA", [NCHUNK, TILE_P, CHUNK * WIN], bf16,
                            kind="ExternalInput").ap()
    selA_t = nc.dram_tensor("selA", [NCHUNK, TILE_P, CHUNK * 128], bf16,
                            kind="ExternalInput").ap()
    rfiB_t = nc.dram_tensor("rfiB", [TILE_P, NB_T * 128 // 16], i16,
                            kind="ExternalInput").ap()
    rdiB_t = nc.dram_tensor("rdiB", [TILE_P, NB_T * 128 // 16], i16,
                            kind="ExternalInput").ap()
    mskB_t = nc.dram_tensor("mskB", [TILE_P, NB_T * B_STRIDE], bf16,
                            kind="ExternalInput").ap()
    selB_t = nc.dram_tensor("selB", [TILE_P, NB_T * 128], bf16,
                            kind="ExternalInput").ap()
    rfiS0_t = nc.dram_tensor("rfiS0", [TILE_P, 8], i16,
                             kind="ExternalInput").ap()
    rfiS1_t = nc.dram_tensor("rfiS1", [TILE_P, 8], i16,
                             kind="ExternalInput").ap()
    rdiS_t = nc.dram_tensor("rdiS", [TILE_P, NS // 16], i16,
                            kind="ExternalInput").ap()
    selS_t = nc.dram_tensor("selS", [TILE_P, (NS // 128) * 128], bf16,
                            kind="ExternalInput").ap()
    sidxS_t = nc.dram_tensor("sidxS", [TILE_P, NRANK * NS // 16], i16,
                             kind="ExternalInput").ap()
    oa_t = [nc.dram_tensor(f"oa{p}", [4096 + WIN, C], f32,
                           kind="ExternalOutput").ap() for p in range(4)]
    ob_t = nc.dram_tensor("ob", [4096, C], f32, kind="ExternalOutput").ap()
    oc_t = nc.dram_tensor("oc", [CELLS_PER_CORE + NS, C], f32,
                          kind="ExternalOutput").ap()

    MUL, ADD = mybir.AluOpType.mult, mybir.AluOpType.add
    AXX = mybir.AxisListType.X

    with tile.TileContext(nc) as tc:
        with (
            tc.tile_pool(name="cst", bufs=1) as cst,
            tc.tile_pool(name="seq", bufs=3) as seq,
            tc.tile_pool(name="gp", bufs=3) as gp,
            tc.tile_pool(name="dp", bufs=3) as dp,
            tc.tile_pool(name="sp", bufs=3) as sp,
            tc.tile_pool(name="xp", bufs=2) as xp,
            tc.tile_pool(name="xb", bufs=1) as xb,
            tc.tile_pool(name="ps", bufs=6, space="PSUM") as ps,
            tc.tile_pool(name="psb", bufs=2, space="PSUM") as psb,
        ):
            # ---- B/S inputs ----
            rfiB_sb = cst.tile([TILE_P, NB_T * 128 // 16], i16)
            rdiB_sb = cst.tile([TILE_P, NB_T * 128 // 16], i16)
            mskB_sb = cst.tile([TILE_P, NB_T * B_STRIDE], bf16)
            rfiS0_sb = cst.tile([TILE_P, 8], i16)
            rfiS1_sb = cst.tile([TILE_P, 8], i16)
            rdiS_sb = cst.tile([TILE_P, NS // 16], i16)
            selS_sb = cst.tile([TILE_P, (NS // 128) * 128], bf16)
            sidS_sb = cst.tile([TILE_P, NRANK * NS // 16], i16)
            nc.sync.dma_start(rfiB_sb[:], rfiB_t)
            nc.sync.dma_start(rdiB_sb[:], rdiB_t)
            nc.sync.dma_start(mskB_sb[:], mskB_t)
            nc.sync.dma_start(rfiS0_sb[:], rfiS0_t)
            nc.sync.dma_start(rfiS1_sb[:], rfiS1_t)
            nc.sync.dma_start(rdiS_sb[:], rdiS_t)
            nc.sync.dma_start(selS_sb[:], selS_t)
            nc.sync.dma_start(sidS_sb[:], sidxS_t)
            gB_sb = cst.tile([TILE_P, NB_T * C // 2], f32)
            dbB_sb = cst.tile([TILE_P, NB_T * 64], f32)
            gS_sb = cst.tile([TILE_P, (NS // 128) * C // 2], f32)
            dbS_sb = cst.tile([TILE_P, (NS // 128) * 64], f32)
            dB_sb = cst.tile([TILE_P, NB_T], bf16)
            adB_sb = cst.tile([TILE_P, NB_T * B_STRIDE], bf16)
            dS_sb = cst.tile([TILE_P, NS // 128], bf16)
            gsS_sb = cst.tile([TILE_P, (NS // 128) * C], f32)

            def extract(sel_sb, db2, d_out, njw):
                """d_out[p, j] = sum_k sel[p,j,k]*db[p,j,k]  (njw j's)."""
                nw = njw * 128
                nc.vector.tensor_tensor(
                    out=sel_sb[:, :nw], in0=sel_sb[:, :nw], in1=db2, op=MUL)
                p3 = sel_sb[:, :nw].rearrange("p (j e) -> p j e", e=128)
                w = 64
                while w >= 8:
                    nc.vector.tensor_tensor(
                        out=p3[:, :, :w], in0=p3[:, :, :w],
                        in1=p3[:, :, w:2 * w], op=ADD)
                    w //= 2
                with nc.allow_low_precision(reason="one-hot, exact"):
                    nc.vector.tensor_reduce(
                        out=d_out, in_=p3[:, :, :8], axis=AXX, op=ADD)

            # ---- stream A ----
            for c in range(NCHUNK):
                rfi_sb = seq.tile([TILE_P, NI // 16], i16, tag="rfi")
                rdi_sb = seq.tile([TILE_P, NI // 16], i16, tag="rdi")
                msk_sb = seq.tile([TILE_P, CHUNK * WIN], bf16, tag="msk")
                nc.sync.dma_start(rfi_sb[:], rfiA_t[c])
                nc.sync.dma_start(rdi_sb[:], rdiA_t[c])
                nc.sync.dma_start(msk_sb[:], mskA_t[c])
                sel_sbs = []
                for h in range(2):
                    s_sb = sp.tile([TILE_P, NI // 2], bf16, tag="sel")
                    nc.sync.dma_start(
                        s_sb[:],
                        selA_t[c][:, h * (NI // 2):(h + 1) * (NI // 2)])
                    sel_sbs.append(s_sb)

                g_sb = gp.tile([TILE_P, CHUNK * C // 2], f32, tag="g")
                db_sb = gp.tile([TILE_P, CHUNK * 64], f32, tag="db")
                g3f = g_sb[:].rearrange("p (j e) -> p j e", e=C // 2)
                db3f = db_sb[:].rearrange("p (j e) -> p j e", e=64)
                HN = NI // 2
                nc.gpsimd.dma_gather(g3f[:, :HALF, :], feat_t,
                                     rfi_sb[:, :HN // 16], HN, HN, C // 2,
                                     single_packet=False, queue_num=c % 4)
                nc.gpsimd.dma_gather(g3f[:, HALF:, :], feat_t,
                                     rfi_sb[:, HN // 16:], HN, HN, C // 2,
                                     single_packet=False,
                                     queue_num=(c + 1) % 4)
                nc.gpsimd.dma_gather(db3f[:, :HALF, :], dep_t,
                                     rdi_sb[:, :HN // 16], HN, HN, 64,
                                     single_packet=False,
                                     queue_num=(c + 2) % 4)
                nc.gpsimd.dma_gather(db3f[:, HALF:, :], dep_t,
                                     rdi_sb[:, HN // 16:], HN, HN, 64,
                                     single_packet=False,
                                     queue_num=(c + 3) % 4)
                if c == 6:
                    gB3f = gB_sb[:].rearrange("p (j e) -> p j e", e=C // 2)
                    dbB3f = dbB_sb[:].rearrange("p (j e) -> p j e", e=64)
                    NBI = NB_T * 128
                    nc.gpsimd.dma_gather(gB3f, feat_t[A_LIM:, :],
                                         rfiB_sb[:], NBI, NBI, C // 2,
                                         single_packet=False, queue_num=0)
                if c == 7:
                    nc.gpsimd.dma_gather(dbB3f, dep_t, rdiB_sb[:],
                                         NBI, NBI, 64,
                                         single_packet=False, queue_num=1)
                if c == 12:
                    gS3f = gS_sb[:].rearrange("p (j e) -> p j e", e=C // 2)
                    dbS3f = dbS_sb[:].rearrange("p (j e) -> p j e", e=64)
                    nc.gpsimd.dma_gather(gS3f[:, 0:1, :], feat_t,
                                         rfiS0_sb[:], 128, 128, C // 2,
                                         single_packet=False, queue_num=2)
                    nc.gpsimd.dma_gather(gS3f[:, 1:2, :], feat_t[A_LIM:, :],
                                         rfiS1_sb[:], 128, 128, C // 2,
                                         single_packet=False, queue_num=3)
                    nc.gpsimd.dma_gather(dbS3f, dep_t, rdiS_sb[:],
                                         NS, NS, 64, single_packet=False,
                                         queue_num=2)

                g3 = g_sb[:].bitcast(bf16).rearrange("p (j e) -> p j e", e=C)
                db2 = db_sb[:].bitcast(bf16)

                d_sb = dp.tile([TILE_P, CHUNK], bf16, tag="d")
                for h in range(2):
                    extract(sel_sbs[h],
                            db2[:, h * (NI // 2):(h + 1) * (NI // 2)],
                            d_sb[:, h * HALF:(h + 1) * HALF], HALF)

                ad_sb = dp.tile([TILE_P, CHUNK * WIN], bf16, tag="ad")
                ad3 = ad_sb[:].rearrange("p (j w) -> p j w", w=WIN)
                nc.vector.tensor_tensor(
                    out=ad3,
                    in0=msk_sb[:].rearrange("p (j w) -> p j w", w=WIN),
                    in1=d_sb[:].to_broadcast([TILE_P, CHUNK, WIN]), op=MUL)

                # matmuls by phase; tile j = phi + 4u covers output rows
                # [256c + 4phi + 16u, +16) of slab oa[phi]
                for phi in range(4):
                    tmp_sb = xp.tile([WIN, 16 * C], f32, tag="tmp")
                    for gq in range(4):
                        pt = ps.tile([WIN, 4 * C], f32, tag="pt",
                                     space="PSUM")
                        for m in range(4):
                            j = phi + 4 * (4 * gq + m)
                            nc.tensor.matmul(out=pt[:, C * m:C * (m + 1)],
                                             lhsT=ad3[:, j, :],
                                             rhs=g3[:, j, :],
                                             start=True, stop=True)
                        nc.scalar.copy(
                            tmp_sb[:, 4 * C * gq:4 * C * (gq + 1)], pt[:])
                    r0 = 256 * c + 4 * phi
                    dst = oa_t[phi][r0:r0 + 256, :].rearrange(
                        "(u w) e -> w u e", u=16)
                    nc.scalar.dma_start(
                        dst, tmp_sb[:].rearrange("w (u e) -> w u e", e=C))

                if c == 9:
                    # B extraction mid-stream
                    dbB2 = dbB_sb[:].bitcast(bf16)
                    selB_sb = sp.tile([TILE_P, NI // 2], bf16, tag="sel")
                    nc.sync.dma_start(selB_sb[:], selB_t[:, :NB_T * 128])
                    extract(selB_sb, dbB2[:], dB_sb[:], NB_T)
                if c == 10:
                    gB3 = gB_sb[:].bitcast(bf16).rearrange(
                        "p (j e) -> p j e", e=C)
                    adB3 = adB_sb[:].rearrange("p (j w) -> p j w",
                                               w=B_STRIDE)
                    nc.vector.tensor_tensor(
                        out=adB3,
                        in0=mskB_sb[:].rearrange("p (j w) -> p j w",
                                                 w=B_STRIDE),
                        in1=dB_sb[:].to_broadcast([TILE_P, NB_T, B_STRIDE]),
                        op=MUL)
                if c in (11, 12):
                    # B matmuls: 16 tiles -> ob rows [2048b, 2048b+2048)
                    bblk = c - 11
                    gB3 = gB_sb[:].bitcast(bf16).rearrange(
                        "p (j e) -> p j e", e=C)
                    adB3 = adB_sb[:].rearrange("p (j w) -> p j w",
                                               w=B_STRIDE)
                    tmpB_sb = xb.tile([TILE_P, 16 * C], f32, tag="tmpB")
                    for gq in range(4):
                        ptB = psb.tile([TILE_P, 4 * C], f32, tag="ptB",
                                       space="PSUM")
                        for m in range(4):
                            j = 16 * bblk + 4 * gq + m
                            nc.tensor.matmul(out=ptB[:, C * m:C * (m + 1)],
                                             lhsT=adB3[:, j, :],
                                             rhs=gB3[:, j, :],
                                             start=True, stop=True)
                        nc.scalar.copy(
                            tmpB_sb[:, 4 * C * gq:4 * C * (gq + 1)], ptB[:])
                    dst = ob_t[2048 * bblk:2048 * (bblk + 1), :].rearrange(
                        "(u w) e -> w u e", u=16)
                    nc.scalar.dma_start(
                        dst, tmpB_sb[:].rearrange("w (u e) -> w u e", e=C))

                if c == 14:
                    # spill products
                    dbS2 = dbS_sb[:].bitcast(bf16)
                    extract(selS_sb, dbS2[:], dS_sb[:], NS // 128)
                    gS3 = gS_sb[:].bitcast(bf16).rearrange(
                        "p (j e) -> p j e", e=C)
                    gsS3 = gsS_sb[:].rearrange("p (j e) -> p j e", e=C)
                    nc.vector.tensor_tensor(
                        out=gsS3, in0=gS3,
                        in1=dS_sb[:].to_broadcast(
                            [TILE_P, NS // 128, C]), op=MUL)

            # spill scatters (chained: call r holds rank-r duplicate cells)
            gsS3 = gsS_sb[:].rearrange("p (j e) -> p j e", e=C)
            prev = None
            for r in range(NRANK):
                sc = nc.gpsimd.dma_scatter_add(
                    oc_t, gsS3,
                    sidS_sb[:, r * NS // 16:(r + 1) * NS // 16],
                    NS, NS, C, single_packet=False, queue_num=r % 4)
                if prev is not None:
                    add_dep_helper(sc.ins, prev.ins, reason="spill chain")
                prev = sc
    nc.compile()
    return nc


def _get_program():
    if "nc" not in _CACHED:
        _CACHED["nc"] = _build_program()
    return _CACHED["nc"]


# ---------------------------------------------------------------- entry
def kernel(depth, feat, ranks_depth, ranks_feat, ranks_bev,
           interval_starts=None, interval_lengths=None):
    from concourse import bass_utils

    depth = np.asarray(depth, dtype=np.float32)
    feat = np.asarray(feat, dtype=np.float32)
    feat_flat = _bf16(feat.transpose(0, 1, 3, 4, 2).reshape(-1, C)) \
        .view(np.float32)
    dep_blk = _bf16(depth.reshape(N_DEP_BLK, 128)).view(np.float32)

    cores = _preprocess(ranks_depth, ranks_feat, ranks_bev)
    in_maps = []
    for k in range(NCORES):
        cd = cores[k]
        in_maps.append({
            "feat_tbl": feat_flat, "dep_tbl": dep_blk,
            "rfiA": cd["rfiA"], "rdiA": cd["rdiA"], "mskA": cd["mskA"],
            "selA": cd["selA"], "rfiB": cd["rfiB"], "rdiB": cd["rdiB"],
            "mskB": cd["mskB"], "selB": cd["selB"],
            "rfiS0": cd["rfiS0"], "rfiS1": cd["rfiS1"],
            "rdiS": cd["rdiS"], "selS": cd["selS"], "sidxS": cd["sidxS"],
        })

    nc = _get_program()
    res = bass_utils.run_bass_kernel_spmd(nc, in_maps,
                                          core_ids=list(range(NCORES)))
    _CACHED["last_results"] = res

    out_full = np.zeros((B, C, 1, 128, 128), np.float32)
    for k in range(NCORES):
        r = res.results[k]
        oc = sum(np.asarray(r[f"oa{p}"])[:CELLS_PER_CORE] for p in range(4))
        oc = oc + np.asarray(r["ob"]) + np.asarray(r["oc"])[:CELLS_PER_CORE]
        b, blk = k // 4, k % 4
        out_full[b, :, 0, 32 * blk:32 * (blk + 1), :] = \
            oc.T.reshape(C, 32, 128)
    return out_full


# revision 25
# speedup vs baseline: 1.1857x; 1.0144x over previous
"""BevPoolV2 Trainium2 kernel (8 NeuronCores, SPMD, no collectives).

v6: fixed windows.  Stream A uses 1024 tiles with static windows
[4t, 4t+16); a point in cell x may sit in any of 4 candidate tiles
(greedy assignment on host), the mask encodes its window column.  Tiles
of phase t%4 write disjoint row ranges of phase slab oa<phi> with plain
streaming DMA -- no scatter, no RMW, no output descriptor generation.
Stream B (feat rows >= 32768, int16 gather-index limit) uses 64 tiles
with windows [64t, 64t+64) -> slab ob.  The handful of points that
overflow their 4 candidate tiles go to a 256-slot spill stream with
chained dma_scatter_add calls -> slab oc.  Host sums the six slabs.
Depth extraction: host ships one-hot lane masks (sel); DVE does mult
(2x bf16) + binary-tree adds + an 8-wide reduce.  Gathers spread over
all 4 SWDGE queues (Q7 core pairs).
"""
import numpy as np

B, N, D, H, W = 2, 6, 120, 32, 88
C = 128
NCELLS = 32768
NCORES = 8
CELLS_PER_CORE = NCELLS // NCORES   # 4096
TILE_P = 128
STRIDE, WIN = 4, 16                 # A window geometry
T_A = 1024                          # A tiles
CHUNK = 64                          # tiles per A-chunk
NCHUNK = 16
HALF = 32
NI = CHUNK * TILE_P                 # 8192 idxs per chunk
B_STRIDE = 128                      # B window width (= stride)
NB_T = 32                           # B tiles
NS = 256                            # spill slots (2 tiles)
NRANK = 8                           # chained spill scatter calls
DUMMY = CELLS_PER_CORE              # trash row in oc
N_FEAT_ROWS = B * N * H * W         # 33792
N_DEPTH = B * N * D * H * W         # 4055040
N_DEP_BLK = N_DEPTH // 128          # 31680
A_LIM = 32768


def _pack16(ent):
    """entry i -> int16 storage [i%16, i//16], replicated to 128 partitions."""
    a = np.asarray(ent, np.int16).reshape(-1, 16).T
    return np.ascontiguousarray(np.tile(a, (8, 1)))


def _bf16(x):
    import ml_dtypes
    return np.ascontiguousarray(np.asarray(x).astype(ml_dtypes.bfloat16))


# ---------------------------------------------------------------- host prep
def _assign_fixed(rb, n_tiles, stride, win):
    """Greedy earliest-tile assignment of sorted cells to fixed windows
    [stride*t, stride*t+win).  Returns tile_id per point (-1 = spill)."""
    fill = np.zeros(n_tiles, np.int32)
    tile_id = np.full(len(rb), -1, np.int64)
    cells, starts, counts = np.unique(rb, return_index=True,
                                      return_counts=True)
    for x, s0, n in zip(cells, starts, counts):
        t_hi = int(x) // stride
        t_lo = max(0, t_hi - (win // stride - 1))
        left = int(n)
        for t in range(t_lo, min(t_hi, n_tiles - 1) + 1):
            take = min(left, 128 - fill[t])
            if take > 0:
                i0 = s0 + n - left
                tile_id[i0:i0 + take] = t
                fill[t] += take
                left -= take
            if left == 0:
                break
    return tile_id


def _stream_arrays(rb, rf, rd, tile_id, n_tiles, stride, win):
    """Pack per-point data into [n_tiles, 128] slot arrays + masks/sel."""
    keep = tile_id >= 0
    rbk, rfk, rdk, tk = rb[keep], rf[keep], rd[keep], tile_id[keep]
    order = np.argsort(tk, kind="stable")
    rbk, rfk, rdk, tk = rbk[order], rfk[order], rdk[order], tk[order]
    fill = np.bincount(tk, minlength=n_tiles)
    t_start = np.concatenate([[0], np.cumsum(fill)])
    slot = np.arange(len(tk)) - t_start[tk]
    rf_t = np.zeros((n_tiles, TILE_P), np.int64)
    rd_t = np.zeros((n_tiles, TILE_P), np.int64)
    msk = np.zeros((n_tiles, TILE_P, win), np.float32)
    sel = np.zeros((n_tiles, TILE_P, 128), np.float32)
    rf_t[tk, slot] = rfk
    rd_t[tk, slot] = rdk
    msk[tk, slot, rbk - stride * tk] = 1.0
    sel[tk, slot, rdk % 128] = 1.0
    return rf_t, rd_t, msk, sel


def _preprocess(ranks_depth, ranks_feat, ranks_bev):
    ranks_bev = np.asarray(ranks_bev)
    ranks_feat = np.asarray(ranks_feat).astype(np.int64)
    ranks_depth = np.asarray(ranks_depth).astype(np.int64)
    bounds = np.searchsorted(ranks_bev, np.arange(0, NCELLS + 1, CELLS_PER_CORE))
    cores = []
    for k in range(NCORES):
        lo, hi = int(bounds[k]), int(bounds[k + 1])
        rb = ranks_bev[lo:hi].astype(np.int64) - k * CELLS_PER_CORE
        rf = ranks_feat[lo:hi]
        rd = ranks_depth[lo:hi]
        isB = rf >= A_LIM

        # ---------------- stream A ----------------
        rbA, rfA, rdA = rb[~isB], rf[~isB], rd[~isB]
        tidA = _assign_fixed(rbA, T_A, STRIDE, WIN)
        rfA_t, rdA_t, mskA_f, selA_f = _stream_arrays(
            rbA, rfA, rdA, tidA, T_A, STRIDE, WIN)
        rfiA = np.empty((NCHUNK, TILE_P, NI // 16), np.int16)
        rdiA = np.empty_like(rfiA)
        mskA = np.empty((NCHUNK, TILE_P, CHUNK * WIN), np.float32)
        selA = np.empty((NCHUNK, TILE_P, CHUNK * 128), np.float32)
        for c in range(NCHUNK):
            t0 = c * CHUNK
            rfiA[c] = _pack16(rfA_t[t0:t0 + CHUNK].reshape(-1))
            rdiA[c] = _pack16((rdA_t[t0:t0 + CHUNK] // 128).reshape(-1))
            mskA[c] = mskA_f[t0:t0 + CHUNK].transpose(1, 0, 2).reshape(
                TILE_P, CHUNK * WIN)
            selA[c] = selA_f[t0:t0 + CHUNK].transpose(1, 0, 2).reshape(
                TILE_P, CHUNK * 128)

        # ---------------- stream B ----------------
        rbB, rfB, rdB = rb[isB], rf[isB], rd[isB]
        tidB = _assign_fixed(rbB, NB_T, B_STRIDE, B_STRIDE)
        rfB_t, rdB_t, mskB_f, selB_f = _stream_arrays(
            rbB, rfB - A_LIM, rdB, tidB, NB_T, B_STRIDE, B_STRIDE)
        rfiB = _pack16(rfB_t.reshape(-1))
        rdiB = _pack16((rdB_t // 128).reshape(-1))
        mskB = mskB_f.transpose(1, 0, 2).reshape(TILE_P, NB_T * B_STRIDE)
        selB = selB_f.transpose(1, 0, 2).reshape(TILE_P, NB_T * 128)

        # ---------------- spill stream ----------------
        # tile 0: A spills (full feat table); tile 1: B spills (B table)
        spA, spB = tidA < 0, tidB < 0
        nSA, nSB = int(spA.sum()), int(spB.sum())
        assert nSA <= 128 and nSB <= 128, (k, nSA, nSB)
        rbS = np.concatenate([rbA[spA], np.full(128 - nSA, -1),
                              rbB[spB], np.full(128 - nSB, -1)])
        rfS0 = np.zeros(128, np.int64)
        rfS0[:nSA] = rfA[spA]
        rfS1 = np.zeros(128, np.int64)
        rfS1[:nSB] = rfB[spB] - A_LIM
        rdS_e = np.zeros(NS, np.int64)
        rdS_e[:nSA] = rdA[spA]
        rdS_e[128:128 + nSB] = rdB[spB]
        rfiS0 = _pack16(rfS0)
        rfiS1 = _pack16(rfS1)
        rdiS = _pack16(rdS_e // 128)
        # slot i -> partition i%128, block i//128 (gather row layout)
        selS = np.zeros((TILE_P, (NS // 128) * 128), np.float32)
        live = rbS >= 0
        i_s = np.arange(NS)[live]
        selS[i_s % 128, (i_s // 128) * 128 + (rdS_e[live] % 128)] = 1.0
        # scatter entries: NRANK chained calls (dup cells split by rank);
        # dead entries go to per-entry trash rows so their RMWs parallelize
        sidxS = np.tile(DUMMY + np.arange(NS), (NRANK, 1))
        seen = {}
        for i in np.nonzero(live)[0]:
            cell = int(rbS[i])
            r = seen.get(cell, 0)
            assert r < NRANK, (k, cell)
            sidxS[r, i] = cell
            seen[cell] = r + 1
        sidxS_p = np.concatenate([_pack16(sidxS[r]) for r in range(NRANK)],
                                 axis=1)

        cores.append(dict(rfiA=rfiA, rdiA=rdiA, mskA=_bf16(mskA),
                          selA=_bf16(selA), rfiB=rfiB, rdiB=rdiB,
                          mskB=_bf16(mskB), selB=_bf16(selB),
                          rfiS0=rfiS0, rfiS1=rfiS1, rdiS=rdiS,
                          selS=_bf16(selS), sidxS=sidxS_p))
    return cores


# ---------------------------------------------------------------- program
_CACHED = {}


def _build_program():
    import concourse.bass as bass
    import concourse.bacc as bacc
    import concourse.tile as tile
    from concourse import mybir
    from concourse.tile import add_dep_helper

    nc = bacc.Bacc("TRN2", target_bir_lowering=False, debug=False,
                   num_swdge_queues=4)
    f32, bf16, i16 = mybir.dt.float32, mybir.dt.bfloat16, mybir.dt.int16
    feat_t = nc.dram_tensor("feat_tbl", [N_FEAT_ROWS, C // 2], f32,
                            kind="ExternalInput").ap()
    dep_t = nc.dram_tensor("dep_tbl", [N_DEP_BLK, 64], f32,
                           kind="ExternalInput").ap()
    rfiA_t = nc.dram_tensor("rfiA", [NCHUNK, TILE_P, NI // 16], i16,
                            kind="ExternalInput").ap()
    rdiA_t = nc.dram_tensor("rdiA", [NCHUNK, TILE_P, NI // 16], i16,
                            kind="ExternalInput").ap()
    mskA_t = nc.dram_tensor("mskA", [NCHUNK, TILE_P, CHUNK * WIN], bf16,
                            kind="ExternalInput").ap()
    selA_t = nc.dram_tensor("selA", [NCHUNK, TILE_P, CHUNK * 128], bf16,
                            kind="ExternalInput").ap()
    rfiB_t = nc.dram_tensor("rfiB", [TILE_P, NB_T * 128 // 16], i16,
                            kind="ExternalInput").ap()
    rdiB_t = nc.dram_tensor("rdiB", [TILE_P, NB_T * 128 // 16], i16,
                            kind="ExternalInput").ap()
    mskB_t = nc.dram_tensor("mskB", [TILE_P, NB_T * B_STRIDE], bf16,
                            kind="ExternalInput").ap()
    selB_t = nc.dram_tensor("selB", [TILE_P, NB_T * 128], bf16,
                            kind="ExternalInput").ap()
    rfiS0_t = nc.dram_tensor("rfiS0", [TILE_P, 8], i16,
                             kind="ExternalInput").ap()
    rfiS1_t = nc.dram_tensor("rfiS1", [TILE_P, 8], i16,
                             kind="ExternalInput").ap()
    rdiS_t = nc.dram_tensor("rdiS", [TILE_P, NS // 16], i16,
                            kind="ExternalInput").ap()
    selS_t = nc.dram_tensor("selS", [TILE_P, (NS // 128) * 128], bf16,
                            kind="ExternalInput").ap()
    sidxS_t = nc.dram_tensor("sidxS", [TILE_P, NRANK * NS // 16], i16,
                             kind="ExternalInput").ap()
    oa_t = [nc.dram_tensor(f"oa{p}", [4096 + WIN, C], f32,
                           kind="ExternalOutput").ap() for p in range(4)]
    ob_t = nc.dram_tensor("ob", [4096, C], f32, kind="ExternalOutput").ap()
    oc_t = [nc.dram_tensor(f"oc{r}", [CELLS_PER_CORE + NS, C], f32,
                           kind="ExternalOutput").ap() for r in range(NRANK)]

    MUL, ADD = mybir.AluOpType.mult, mybir.AluOpType.add
    AXX = mybir.AxisListType.X

    with tile.TileContext(nc) as tc:
        with (
            tc.tile_pool(name="cst", bufs=1) as cst,
            tc.tile_pool(name="seq", bufs=3) as seq,
            tc.tile_pool(name="gp", bufs=3) as gp,
            tc.tile_pool(name="dp", bufs=3) as dp,
            tc.tile_pool(name="sp", bufs=3) as sp,
            tc.tile_pool(name="xp", bufs=2) as xp,
            tc.tile_pool(name="xb", bufs=1) as xb,
            tc.tile_pool(name="ps", bufs=6, space="PSUM") as ps,
            tc.tile_pool(name="psb", bufs=2, space="PSUM") as psb,
        ):
            # ---- B/S inputs ----
            rfiB_sb = cst.tile([TILE_P, NB_T * 128 // 16], i16)
            rdiB_sb = cst.tile([TILE_P, NB_T * 128 // 16], i16)
            mskB_sb = cst.tile([TILE_P, NB_T * B_STRIDE], bf16)
            rfiS0_sb = cst.tile([TILE_P, 8], i16)
            rfiS1_sb = cst.tile([TILE_P, 8], i16)
            rdiS_sb = cst.tile([TILE_P, NS // 16], i16)
            selS_sb = cst.tile([TILE_P, (NS // 128) * 128], bf16)
            sidS_sb = cst.tile([TILE_P, NRANK * NS // 16], i16)
            nc.sync.dma_start(rfiB_sb[:], rfiB_t)
            nc.sync.dma_start(rdiB_sb[:], rdiB_t)
            nc.sync.dma_start(mskB_sb[:], mskB_t)
            nc.sync.dma_start(rfiS0_sb[:], rfiS0_t)
            nc.sync.dma_start(rfiS1_sb[:], rfiS1_t)
            nc.sync.dma_start(rdiS_sb[:], rdiS_t)
            nc.sync.dma_start(selS_sb[:], selS_t)
            nc.sync.dma_start(sidS_sb[:], sidxS_t)
            gB_sb = cst.tile([TILE_P, NB_T * C // 2], f32)
            dbB_sb = cst.tile([TILE_P, NB_T * 64], f32)
            gS_sb = cst.tile([TILE_P, (NS // 128) * C // 2], f32)
            dbS_sb = cst.tile([TILE_P, (NS // 128) * 64], f32)
            dB_sb = cst.tile([TILE_P, NB_T], bf16)
            adB_sb = cst.tile([TILE_P, NB_T * B_STRIDE], bf16)
            dS_sb = cst.tile([TILE_P, NS // 128], bf16)
            gsS_sb = cst.tile([TILE_P, (NS // 128) * C], f32)

            def extract(sel_sb, db2, d_out, njw):
                """d_out[p, j] = sum_k sel[p,j,k]*db[p,j,k]  (njw j's)."""
                nw = njw * 128
                nc.vector.tensor_tensor(
                    out=sel_sb[:, :nw], in0=sel_sb[:, :nw], in1=db2, op=MUL)
                p3 = sel_sb[:, :nw].rearrange("p (j e) -> p j e", e=128)
                w = 64
                while w >= 8:
                    nc.vector.tensor_tensor(
                        out=p3[:, :, :w], in0=p3[:, :, :w],
                        in1=p3[:, :, w:2 * w], op=ADD)
                    w //= 2
                with nc.allow_low_precision(reason="one-hot, exact"):
                    nc.vector.tensor_reduce(
                        out=d_out, in_=p3[:, :, :8], axis=AXX, op=ADD)

            # ---- stream A ----
            for c in range(NCHUNK):
                rfi_sb = seq.tile([TILE_P, NI // 16], i16, tag="rfi")
                rdi_sb = seq.tile([TILE_P, NI // 16], i16, tag="rdi")
                msk_sb = seq.tile([TILE_P, CHUNK * WIN], bf16, tag="msk")
                nc.sync.dma_start(rfi_sb[:], rfiA_t[c])
                nc.sync.dma_start(rdi_sb[:], rdiA_t[c])
                nc.sync.dma_start(msk_sb[:], mskA_t[c])
                sel_sbs = []
                for h in range(2):
                    s_sb = sp.tile([TILE_P, NI // 2], bf16, tag="sel")
                    nc.sync.dma_start(
                        s_sb[:],
                        selA_t[c][:, h * (NI // 2):(h + 1) * (NI // 2)])
                    sel_sbs.append(s_sb)

                g_sb = gp.tile([TILE_P, CHUNK * C // 2], f32, tag="g")
                db_sb = gp.tile([TILE_P, CHUNK * 64], f32, tag="db")
                g3f = g_sb[:].rearrange("p (j e) -> p j e", e=C // 2)
                db3f = db_sb[:].rearrange("p (j e) -> p j e", e=64)
                HN = NI // 2
                nc.gpsimd.dma_gather(g3f[:, :HALF, :], feat_t,
                                     rfi_sb[:, :HN // 16], HN, HN, C // 2,
                                     single_packet=False, queue_num=c % 4)
                nc.gpsimd.dma_gather(g3f[:, HALF:, :], feat_t,
                                     rfi_sb[:, HN // 16:], HN, HN, C // 2,
                                     single_packet=False,
                                     queue_num=(c + 1) % 4)
                nc.gpsimd.dma_gather(db3f[:, :HALF, :], dep_t,
                                     rdi_sb[:, :HN // 16], HN, HN, 64,
                                     single_packet=False,
                                     queue_num=(c + 2) % 4)
                nc.gpsimd.dma_gather(db3f[:, HALF:, :], dep_t,
                                     rdi_sb[:, HN // 16:], HN, HN, 64,
                                     single_packet=False,
                                     queue_num=(c + 3) % 4)
                if c == 6:
                    gB3f = gB_sb[:].rearrange("p (j e) -> p j e", e=C // 2)
                    dbB3f = dbB_sb[:].rearrange("p (j e) -> p j e", e=64)
                    NBI = NB_T * 128
                    nc.gpsimd.dma_gather(gB3f, feat_t[A_LIM:, :],
                                         rfiB_sb[:], NBI, NBI, C // 2,
                                         single_packet=False, queue_num=0)
                if c == 7:
                    nc.gpsimd.dma_gather(dbB3f, dep_t, rdiB_sb[:],
                                         NBI, NBI, 64,
                                         single_packet=False, queue_num=1)
                if c == 12:
                    gS3f = gS_sb[:].rearrange("p (j e) -> p j e", e=C // 2)
                    dbS3f = dbS_sb[:].rearrange("p (j e) -> p j e", e=64)
                    nc.gpsimd.dma_gather(gS3f[:, 0:1, :], feat_t,
                                         rfiS0_sb[:], 128, 128, C // 2,
                                         single_packet=False, queue_num=2)
                    nc.gpsimd.dma_gather(gS3f[:, 1:2, :], feat_t[A_LIM:, :],
                                         rfiS1_sb[:], 128, 128, C // 2,
                                         single_packet=False, queue_num=3)
                    nc.gpsimd.dma_gather(dbS3f, dep_t, rdiS_sb[:],
                                         NS, NS, 64, single_packet=False,
                                         queue_num=2)

                g3 = g_sb[:].bitcast(bf16).rearrange("p (j e) -> p j e", e=C)
                db2 = db_sb[:].bitcast(bf16)

                d_sb = dp.tile([TILE_P, CHUNK], bf16, tag="d")
                for h in range(2):
                    extract(sel_sbs[h],
                            db2[:, h * (NI // 2):(h + 1) * (NI // 2)],
                            d_sb[:, h * HALF:(h + 1) * HALF], HALF)

                ad_sb = dp.tile([TILE_P, CHUNK * WIN], bf16, tag="ad")
                ad3 = ad_sb[:].rearrange("p (j w) -> p j w", w=WIN)
                nc.vector.tensor_tensor(
                    out=ad3,
                    in0=msk_sb[:].rearrange("p (j w) -> p j w", w=WIN),
                    in1=d_sb[:].to_broadcast([TILE_P, CHUNK, WIN]), op=MUL)

                # matmuls by phase; tile j = phi + 4u covers output rows
                # [256c + 4phi + 16u, +16) of slab oa[phi]
                for phi in range(4):
                    tmp_sb = xp.tile([WIN, 16 * C], f32, tag="tmp")
                    for gq in range(4):
                        pt = ps.tile([WIN, 4 * C], f32, tag="pt",
                                     space="PSUM")
                        for m in range(4):
                            j = phi + 4 * (4 * gq + m)
                            nc.tensor.matmul(out=pt[:, C * m:C * (m + 1)],
                                             lhsT=ad3[:, j, :],
                                             rhs=g3[:, j, :],
                                             start=True, stop=True)
                        nc.scalar.copy(
                            tmp_sb[:, 4 * C * gq:4 * C * (gq + 1)], pt[:])
                    r0 = 256 * c + 4 * phi
                    dst = oa_t[phi][r0:r0 + 256, :].rearrange(
                        "(u w) e -> w u e", u=16)
                    nc.scalar.dma_start(
                        dst, tmp_sb[:].rearrange("w (u e) -> w u e", e=C))

                if c == 9:
                    # B extraction mid-stream
                    dbB2 = dbB_sb[:].bitcast(bf16)
                    selB_sb = sp.tile([TILE_P, NI // 2], bf16, tag="sel")
                    nc.sync.dma_start(selB_sb[:], selB_t[:, :NB_T * 128])
                    extract(selB_sb, dbB2[:], dB_sb[:], NB_T)
                if c == 10:
                    gB3 = gB_sb[:].bitcast(bf16).rearrange(
                        "p (j e) -> p j e", e=C)
                    adB3 = adB_sb[:].rearrange("p (j w) -> p j w",
                                               w=B_STRIDE)
                    nc.vector.tensor_tensor(
                        out=adB3,
                        in0=mskB_sb[:].rearrange("p (j w) -> p j w",
                                                 w=B_STRIDE),
                        in1=dB_sb[:].to_broadcast([TILE_P, NB_T, B_STRIDE]),
                        op=MUL)
                if c in (11, 12):
                    # B matmuls: 16 tiles -> ob rows [2048b, 2048b+2048)
                    bblk = c - 11
                    gB3 = gB_sb[:].bitcast(bf16).rearrange(
                        "p (j e) -> p j e", e=C)
                    adB3 = adB_sb[:].rearrange("p (j w) -> p j w",
                                               w=B_STRIDE)
                    tmpB_sb = xb.tile([TILE_P, 16 * C], f32, tag="tmpB")
                    for gq in range(4):
                        ptB = psb.tile([TILE_P, 4 * C], f32, tag="ptB",
                                       space="PSUM")
                        for m in range(4):
                            j = 16 * bblk + 4 * gq + m
                            nc.tensor.matmul(out=ptB[:, C * m:C * (m + 1)],
                                             lhsT=adB3[:, j, :],
                                             rhs=gB3[:, j, :],
                                             start=True, stop=True)
                        nc.scalar.copy(
                            tmpB_sb[:, 4 * C * gq:4 * C * (gq + 1)], ptB[:])
                    dst = ob_t[2048 * bblk:2048 * (bblk + 1), :].rearrange(
                        "(u w) e -> w u e", u=16)
                    nc.scalar.dma_start(
                        dst, tmpB_sb[:].rearrange("w (u e) -> w u e", e=C))

                if c == 14:
                    # spill products
                    dbS2 = dbS_sb[:].bitcast(bf16)
                    extract(selS_sb, dbS2[:], dS_sb[:], NS // 128)
                    gS3 = gS_sb[:].bitcast(bf16).rearrange(
                        "p (j e) -> p j e", e=C)
                    gsS3 = gsS_sb[:].rearrange("p (j e) -> p j e", e=C)
                    nc.vector.tensor_tensor(
                        out=gsS3, in0=gS3,
                        in1=dS_sb[:].to_broadcast(
                            [TILE_P, NS // 128, C]), op=MUL)

            # spill scatters: one slab per rank, unchained (no dup cells
            # within a call; cross-rank dups live in different slabs)
            gsS3 = gsS_sb[:].rearrange("p (j e) -> p j e", e=C)
            for r in range(NRANK):
                nc.gpsimd.dma_scatter_add(
                    oc_t[r], gsS3,
                    sidS_sb[:, r * NS // 16:(r + 1) * NS // 16],
                    NS, NS, C, single_packet=False, queue_num=r % 4)
    nc.compile()
    return nc


def _get_program():
    if "nc" not in _CACHED:
        _CACHED["nc"] = _build_program()
    return _CACHED["nc"]


# ---------------------------------------------------------------- entry
def kernel(depth, feat, ranks_depth, ranks_feat, ranks_bev,
           interval_starts=None, interval_lengths=None):
    from concourse import bass_utils

    depth = np.asarray(depth, dtype=np.float32)
    feat = np.asarray(feat, dtype=np.float32)
    feat_flat = _bf16(feat.transpose(0, 1, 3, 4, 2).reshape(-1, C)) \
        .view(np.float32)
    dep_blk = _bf16(depth.reshape(N_DEP_BLK, 128)).view(np.float32)

    cores = _preprocess(ranks_depth, ranks_feat, ranks_bev)
    in_maps = []
    for k in range(NCORES):
        cd = cores[k]
        in_maps.append({
            "feat_tbl": feat_flat, "dep_tbl": dep_blk,
            "rfiA": cd["rfiA"], "rdiA": cd["rdiA"], "mskA": cd["mskA"],
            "selA": cd["selA"], "rfiB": cd["rfiB"], "rdiB": cd["rdiB"],
            "mskB": cd["mskB"], "selB": cd["selB"],
            "rfiS0": cd["rfiS0"], "rfiS1": cd["rfiS1"],
            "rdiS": cd["rdiS"], "selS": cd["selS"], "sidxS": cd["sidxS"],
        })

    nc = _get_program()
    res = bass_utils.run_bass_kernel_spmd(nc, in_maps,
                                          core_ids=list(range(NCORES)))
    _CACHED["last_results"] = res

    out_full = np.zeros((B, C, 1, 128, 128), np.float32)
    for k in range(NCORES):
        r = res.results[k]
        oc = sum(np.asarray(r[f"oa{p}"])[:CELLS_PER_CORE] for p in range(4))
        oc = oc + np.asarray(r["ob"])
        for rr in range(NRANK):
            oc = oc + np.asarray(r[f"oc{rr}"])[:CELLS_PER_CORE]
        b, blk = k // 4, k % 4
        out_full[b, :, 0, 32 * blk:32 * (blk + 1), :] = \
            oc.T.reshape(C, 32, 128)
    return out_full


# revision 34
# speedup vs baseline: 1.4629x; 1.2338x over previous
"""BevPoolV2 Trainium2 kernel (8 NeuronCores, SPMD, no collectives).

v4: multi-queue SWDGE (gathers spread over Q7 core pairs 0-3), batched
DVE depth extraction (is_equal+mult+reduce over half-chunks instead of
per-tile scalar_tensor_tensor), merged scatters (2 per chunk, even/odd
tiles), deeper tile-pool pipelining, B-stream gathers hoisted ahead of
the A loop.
v3: f32-view gathers (bitcast bf16 for compute).
v2: bf16 gather/matmul path, CHUNK=64, psum groups.
Structure: sorted point stream cut at BEV-cell boundaries; core k owns
cells [4096k, 4096(k+1)); window matmuls accumulate into a [4097, 128]
DRAM slab via chained dma_scatter_add; stream B handles feat rows >=
32768 (int16 gather index limit).
"""
import numpy as np

B, N, D, H, W = 2, 6, 120, 32, 88
C = 128
NCELLS = 32768
NCORES = 8
CELLS_PER_CORE = NCELLS // NCORES   # 4096
TILE_P = 128
WIN = 8
CHUNK = 64                          # tiles per A-chunk
NCHUNK = 15
HALF = 32                           # tiles per psum half-chunk
T_A = NCHUNK * CHUNK                # 960 A-tiles
DUMMY = CELLS_PER_CORE              # trash row 4096
N_FEAT_ROWS = B * N * H * W         # 33792
N_DEPTH = B * N * D * H * W         # 4055040
N_DEP_BLK = N_DEPTH // 128          # 31680
A_LIM = 32768                       # feat rows handled by stream A
NB_ROWS = N_FEAT_ROWS - A_LIM       # 1024 rows in stream-B table view
B_CAPS = (21, 9, 3, 1, 1, 1, 1, 1)  # blocks per B scatter call
NB_BLOCKS = sum(B_CAPS)             # 36
NB_SLOTS = NB_BLOCKS * 128          # 4608


def _pack16(ent):
    """entry i -> int16 storage [i%16, i//16], replicated to 128 partitions."""
    a = np.asarray(ent, np.int16).reshape(-1, 16).T
    return np.ascontiguousarray(np.tile(a, (8, 1)))


def _bf16(x):
    import ml_dtypes
    return np.ascontiguousarray(np.asarray(x).astype(ml_dtypes.bfloat16))


# ---------------------------------------------------------------- host prep
def _preprocess(ranks_depth, ranks_feat, ranks_bev):
    ranks_bev = np.asarray(ranks_bev)
    ranks_feat = np.asarray(ranks_feat).astype(np.int64)
    ranks_depth = np.asarray(ranks_depth).astype(np.int64)
    bounds = np.searchsorted(ranks_bev, np.arange(0, NCELLS + 1, CELLS_PER_CORE))
    cores = []
    for k in range(NCORES):
        lo, hi = int(bounds[k]), int(bounds[k + 1])
        rb = ranks_bev[lo:hi].astype(np.int64) - k * CELLS_PER_CORE
        rf = ranks_feat[lo:hi]
        rd = ranks_depth[lo:hi]
        isB = rf >= A_LIM

        # ---------------- stream A ----------------
        rbA, rfA, rdA = rb[~isB], rf[~isB], rd[~isB]
        n = len(rbA)
        assert np.bincount(rbA, minlength=1).max() < 2 * TILE_P
        tiles = []
        s = 0
        while s < n:
            e = min(s + TILE_P, n)
            cut = s + int(np.searchsorted(rbA[s:e], rbA[s] + WIN))
            e = min(e, cut) if cut > s else e
            tiles.append((s, e))
            s = e
        nt = len(tiles)
        assert nt <= T_A, (k, nt)

        rf_i = np.zeros((T_A, TILE_P), np.int64)
        rd_i = np.zeros((T_A, TILE_P), np.int64)
        mask = np.zeros((T_A, TILE_P, WIN), np.float32)
        win0 = np.zeros(T_A, np.int64)
        wid = np.zeros(T_A, np.int64)
        for t, (ts, te) in enumerate(tiles):
            m = te - ts
            rf_i[t, :m] = rfA[ts:te]
            rd_i[t, :m] = rdA[ts:te]
            col = rbA[ts:te] - rbA[ts]
            mask[t, np.arange(m), col] = 1.0
            win0[t] = rbA[ts]
            wid[t] = col[-1] + 1

        rfiA = np.empty((NCHUNK, TILE_P, CHUNK * TILE_P // 16), np.int16)
        rdiA = np.empty_like(rfiA)
        mskA = np.empty((NCHUNK, TILE_P, CHUNK * WIN), np.float32)
        selA = np.zeros((NCHUNK, TILE_P, CHUNK, 128), np.float32)
        ar = np.arange(TILE_P)
        for c in range(NCHUNK):
            t0 = c * CHUNK
            rfiA[c] = _pack16(rf_i[t0:t0 + CHUNK].reshape(-1))
            rdiA[c] = _pack16((rd_i[t0:t0 + CHUNK] // 128).reshape(-1))
            mskA[c] = mask[t0:t0 + CHUNK].transpose(1, 0, 2).reshape(
                TILE_P, CHUNK * WIN)
            rdm_c = (rd_i[t0:t0 + CHUNK] % 128).T  # [P, CHUNK]
            for j in range(CHUNK):
                selA[c, ar, j, rdm_c[:, j]] = 1.0
        selA = selA.reshape(NCHUNK, TILE_P, CHUNK * 128)

        # merged scatter: 2 calls per chunk (even/odd tiles), 256 entries
        # each.  Entry i of call `par`: P=i%128, h=i//128,
        # j = 32*h + 2*(P%16) + par, w = P//16.
        sidxA = np.empty((NCHUNK, TILE_P, 2 * 16), np.int16)
        i_arr = np.arange(256)
        P_arr, h_arr = i_arr % 128, i_arr // 128
        w_arr = P_arr // 16
        for c in range(NCHUNK):
            t0 = c * CHUNK
            for par in range(2):
                j = 32 * h_arr + 2 * (P_arr % 16) + par
                t = t0 + j
                ent = np.where(w_arr < wid[t], win0[t] + w_arr, DUMMY)
                sidxA[c, :, 16 * par:16 * par + 16] = _pack16(ent)
        # scatter calls are unchained: cells must be globally unique per
        # parity slab (adjacent-tile window overlap always crosses parity)
        for par in range(2):
            j = (32 * h_arr[None, :] + 2 * (P_arr[None, :] % 16) + par
                 + CHUNK * np.arange(NCHUNK)[:, None])
            ent = np.where(w_arr[None, :] < wid[j], win0[j] + w_arr[None, :],
                           DUMMY).ravel()
            live = ent[ent != DUMMY]
            assert len(live) == len(np.unique(live)), (k, par)

        # ---------------- stream B ----------------
        rbB, rfB, rdB = rb[isB], rf[isB], rd[isB]
        nB = len(rbB)
        assert nB <= NB_SLOTS, (k, nB)
        starts = np.concatenate([[0], np.cumsum(B_CAPS)]) * 128  # slot starts
        fill = list(starts[:-1])                  # next free slot per bin
        cell_bins = {}                            # cell -> set of bins used
        slot_of = np.full(NB_SLOTS, -1, np.int64)  # slot -> B-point index
        for i in range(nB):
            cell = int(rbB[i])
            used = cell_bins.setdefault(cell, set())
            placed = False
            for o in range(len(B_CAPS)):
                if o in used or fill[o] >= starts[o + 1]:
                    continue
                slot_of[fill[o]] = i
                fill[o] += 1
                used.add(o)
                placed = True
                break
            assert placed, (k, i, cell)
        rfiB_e = np.zeros(NB_SLOTS, np.int64)
        rdiB_e = np.zeros(NB_SLOTS, np.int64)
        selB = np.zeros((TILE_P, NB_BLOCKS, 128), np.float32)
        srowB = np.full(NB_SLOTS, DUMMY, np.int64)
        occ_s = slot_of >= 0
        pi = slot_of[occ_s]
        rfiB_e[occ_s] = rfB[pi] - A_LIM
        rdiB_e[occ_s] = rdB[pi] // 128
        srowB[occ_s] = rbB[pi]
        g = np.arange(NB_SLOTS)
        rdm_flat = np.zeros(NB_SLOTS, np.int64)
        rdm_flat[occ_s] = rdB[pi] % 128
        occ_g = np.zeros(NB_SLOTS, bool)
        occ_g[occ_s] = True
        selB[g[occ_g] % 128, g[occ_g] // 128, rdm_flat[occ_g]] = 1.0
        selB = selB.reshape(TILE_P, NB_BLOCKS * 128)
        rfiB = _pack16(rfiB_e)
        rdiB = _pack16(rdiB_e)
        sidxB = np.empty((TILE_P, NB_BLOCKS * WIN), np.int16)
        for o in range(len(B_CAPS)):
            s0, e0 = starts[o], starts[o + 1]
            sidxB[:, (s0 // 16):(e0 // 16)] = _pack16(srowB[s0:e0])

        cores.append(dict(rfiA=rfiA, rdiA=rdiA, mskA=_bf16(mskA),
                          selA=_bf16(selA), sidxA=sidxA, rfiB=rfiB,
                          rdiB=rdiB, selB=_bf16(selB), sidxB=sidxB))
    return cores


# ---------------------------------------------------------------- program
_CACHED = {}


def _build_program():
    import concourse.bass as bass
    import concourse.bacc as bacc
    import concourse.tile as tile
    from concourse import mybir
    from concourse.tile import add_dep_helper

    nc = bacc.Bacc("TRN2", target_bir_lowering=False, debug=False,
                   num_swdge_queues=4)
    f32, bf16, i16 = mybir.dt.float32, mybir.dt.bfloat16, mybir.dt.int16
    NI = CHUNK * TILE_P                 # 8192 idxs per A chunk
    feat_t = nc.dram_tensor("feat_tbl", [N_FEAT_ROWS, C // 2], f32,
                            kind="ExternalInput").ap()
    dep_t = nc.dram_tensor("dep_tbl", [N_DEP_BLK, 64], f32,
                           kind="ExternalInput").ap()
    rfiA_t = nc.dram_tensor("rfiA", [NCHUNK, TILE_P, NI // 16], i16,
                            kind="ExternalInput").ap()
    rdiA_t = nc.dram_tensor("rdiA", [NCHUNK, TILE_P, NI // 16], i16,
                            kind="ExternalInput").ap()
    mskA_t = nc.dram_tensor("mskA", [NCHUNK, TILE_P, CHUNK * WIN], bf16,
                            kind="ExternalInput").ap()
    selA_t = nc.dram_tensor("selA", [NCHUNK, TILE_P, CHUNK * 128], bf16,
                            kind="ExternalInput").ap()
    sidxA_t = nc.dram_tensor("sidxA", [NCHUNK, TILE_P, 2 * 16], i16,
                             kind="ExternalInput").ap()
    rfiB_t = nc.dram_tensor("rfiB", [TILE_P, NB_SLOTS // 16], i16,
                            kind="ExternalInput").ap()
    rdiB_t = nc.dram_tensor("rdiB", [TILE_P, NB_SLOTS // 16], i16,
                            kind="ExternalInput").ap()
    selB_t = nc.dram_tensor("selB", [TILE_P, NB_BLOCKS * 128], bf16,
                            kind="ExternalInput").ap()
    sidxB_t = nc.dram_tensor("sidxB", [TILE_P, NB_BLOCKS * WIN], i16,
                             kind="ExternalInput").ap()
    out_t = nc.dram_tensor("out", [CELLS_PER_CORE + 1, C], f32,
                           kind="ExternalOutput").ap()
    out2_t = nc.dram_tensor("out2", [CELLS_PER_CORE + 1, C], f32,
                            kind="ExternalOutput").ap()
    out3_t = nc.dram_tensor("out3", [CELLS_PER_CORE + 1, C], f32,
                            kind="ExternalOutput").ap()

    EQ, MUL, ADD = (mybir.AluOpType.is_equal, mybir.AluOpType.mult,
                    mybir.AluOpType.add)
    AXX = mybir.AxisListType.X

    with tile.TileContext(nc) as tc:
        with (
            tc.tile_pool(name="cst", bufs=1) as cst,
            tc.tile_pool(name="seq", bufs=3) as seq,
            tc.tile_pool(name="gp", bufs=2) as gp,
            tc.tile_pool(name="dp", bufs=3) as dp,
            tc.tile_pool(name="sp", bufs=2) as sp,
            tc.tile_pool(name="xp", bufs=2) as xp,
            tc.tile_pool(name="sip", bufs=NCHUNK) as sip,
            tc.tile_pool(name="sg", bufs=NCHUNK) as sg,
            tc.tile_pool(name="ps", bufs=8, space="PSUM") as ps,
        ):
            rfiB_sb = cst.tile([TILE_P, NB_SLOTS // 16], i16)
            rdiB_sb = cst.tile([TILE_P, NB_SLOTS // 16], i16)
            selB_sb = cst.tile([TILE_P, NB_BLOCKS * 128], bf16)
            sidB_sb = cst.tile([TILE_P, NB_BLOCKS * WIN], i16)
            nc.sync.dma_start(rfiB_sb[:], rfiB_t)
            nc.sync.dma_start(rdiB_sb[:], rdiB_t)
            nc.sync.dma_start(selB_sb[:], selB_t)
            nc.sync.dma_start(sidB_sb[:], sidxB_t)
            gB_sb = cst.tile([TILE_P, NB_BLOCKS * C // 2], f32)
            dbB_sb = cst.tile([TILE_P, NB_BLOCKS * 64], f32)

            # ---- stream A ----
            stg_tiles = []
            prev = None
            for c in range(NCHUNK):
                rfi_sb = seq.tile([TILE_P, NI // 16], i16, tag="rfi")
                rdi_sb = seq.tile([TILE_P, NI // 16], i16, tag="rdi")
                msk_sb = seq.tile([TILE_P, CHUNK * WIN], bf16, tag="msk")
                sid_sb = sip.tile([TILE_P, 2 * 16], i16, tag="sid")
                nc.sync.dma_start(rfi_sb[:], rfiA_t[c])
                nc.sync.dma_start(rdi_sb[:], rdiA_t[c])
                nc.sync.dma_start(msk_sb[:], mskA_t[c])
                nc.sync.dma_start(sid_sb[:], sidxA_t[c])
                sel_sbs = []
                for h in range(2):
                    s_sb = sp.tile([TILE_P, NI // 2], bf16, tag="sel")
                    nc.sync.dma_start(
                        s_sb[:],
                        selA_t[c][:, h * (NI // 2):(h + 1) * (NI // 2)])
                    sel_sbs.append(s_sb)

                g_sb = gp.tile([TILE_P, CHUNK * C // 2], f32, tag="g")
                db_sb = gp.tile([TILE_P, CHUNK * 64], f32, tag="db")
                g3f = g_sb[:].rearrange("p (j e) -> p j e", e=C // 2)
                db3f = db_sb[:].rearrange("p (j e) -> p j e", e=64)
                HN = NI // 2
                nc.gpsimd.dma_gather(g3f[:, :HALF, :], feat_t,
                                     rfi_sb[:, :HN // 16], HN, HN, C // 2,
                                     single_packet=False,
                                     queue_num=c % 4)
                nc.gpsimd.dma_gather(g3f[:, HALF:, :], feat_t,
                                     rfi_sb[:, HN // 16:], HN, HN, C // 2,
                                     single_packet=False,
                                     queue_num=(c + 1) % 4)
                nc.gpsimd.dma_gather(db3f[:, :HALF, :], dep_t,
                                     rdi_sb[:, :HN // 16], HN, HN, 64,
                                     single_packet=False,
                                     queue_num=(c + 2) % 4)
                nc.gpsimd.dma_gather(db3f[:, HALF:, :], dep_t,
                                     rdi_sb[:, HN // 16:], HN, HN, 64,
                                     single_packet=False,
                                     queue_num=(c + 3) % 4)
                if c == 5:
                    gB3f = gB_sb[:].rearrange("p (j e) -> p j e", e=C // 2)
                    dbB3f = dbB_sb[:].rearrange("p (j e) -> p j e", e=64)
                    nc.gpsimd.dma_gather(gB3f, feat_t[A_LIM:, :],
                                         rfiB_sb[:], NB_SLOTS, NB_SLOTS,
                                         C // 2, single_packet=False,
                                         queue_num=(c + 1) % 4)
                if c == 6:
                    nc.gpsimd.dma_gather(dbB3f, dep_t, rdiB_sb[:],
                                         NB_SLOTS, NB_SLOTS, 64,
                                         single_packet=False,
                                         queue_num=(c + 2) % 4)

                g3 = g_sb[:].bitcast(bf16).rearrange("p (j e) -> p j e", e=C)
                db2 = db_sb[:].bitcast(bf16)

                # depth extraction: d[p, j] = sum_k sel[p,j,k]*db[p,j,k]
                d_sb = dp.tile([TILE_P, CHUNK], bf16, tag="d")
                for h in range(2):
                    s_sb = sel_sbs[h]
                    nc.vector.tensor_tensor(
                        out=s_sb[:], in0=s_sb[:],
                        in1=db2[:, h * (NI // 2):(h + 1) * (NI // 2)],
                        op=MUL)
                    p3 = s_sb[:].rearrange("p (j e) -> p j e", e=128)
                    w = 64
                    while w >= 8:
                        nc.vector.tensor_tensor(
                            out=p3[:, :, :w], in0=p3[:, :, :w],
                            in1=p3[:, :, w:2 * w], op=ADD)
                        w //= 2
                    with nc.allow_low_precision(reason="one-hot, exact"):
                        nc.vector.tensor_reduce(
                            out=d_sb[:, h * HALF:(h + 1) * HALF],
                            in_=p3[:, :, :8], axis=AXX, op=ADD)

                ad_sb = dp.tile([TILE_P, CHUNK * WIN], bf16, tag="ad")
                ad3 = ad_sb[:].rearrange("p (j w) -> p j w", w=WIN)
                nc.vector.tensor_tensor(
                    out=ad3, in0=msk_sb[:].rearrange("p (j w) -> p j w", w=WIN),
                    in1=d_sb[:].to_broadcast([TILE_P, CHUNK, WIN]), op=MUL)

                # staging: element (w, j2, e) of half h lands at stg
                # partition 16w + j2//2, block 2*(j2%2) + h (flat-order DMA
                # pairing); even tiles end up in blocks 0-1, odd in 2-3.
                stg_sb = sg.tile([TILE_P, 4 * C], f32, tag="stg")
                st4 = stg_sb[:].rearrange("P (b e) -> P b e", e=C)
                for h in range(2):
                    tmp_sb = xp.tile([WIN, HALF * C], f32, tag="tmp")
                    for q in range(HALF // 4):
                        pt = ps.tile([WIN, 4 * C], f32, tag="pt", space="PSUM")
                        for m in range(4):
                            j = HALF * h + 4 * q + m
                            nc.tensor.matmul(out=pt[:, C * m:C * (m + 1)],
                                             lhsT=ad3[:, j, :], rhs=g3[:, j, :],
                                             start=True, stop=True)
                        nc.scalar.copy(tmp_sb[:, 4 * C * q:4 * C * (q + 1)],
                                       pt[:])
                    nc.scalar.dma_start(
                        st4[:, h::2, :],
                        tmp_sb[:].rearrange("w (j e) -> w j e", e=C))
                stg_tiles.append((stg_sb, sid_sb))

            # deferred unchained A scatters: even tiles -> out, odd -> out2
            for c in range(NCHUNK):
                stg_sb, sid_sb = stg_tiles[c]
                st4 = stg_sb[:].rearrange("P (b e) -> P b e", e=C)
                for par in range(2):
                    nc.gpsimd.dma_scatter_add(
                        (out_t, out2_t)[par], st4[:, 2 * par:2 * par + 2, :],
                        sid_sb[:, 16 * par:16 * par + 16],
                        256, 256, C, single_packet=False,
                        queue_num=(2 * c + par) % 4)

            # ---- stream B compute + scatter ----
            gB3 = gB_sb[:].bitcast(bf16).rearrange("p (j e) -> p j e", e=C)
            dbB2 = dbB_sb[:].bitcast(bf16)
            dB_sb = cst.tile([TILE_P, NB_BLOCKS], bf16)
            nc.vector.tensor_tensor(
                out=selB_sb[:], in0=selB_sb[:], in1=dbB2[:], op=MUL)
            pB3 = selB_sb[:].rearrange("p (j e) -> p j e", e=128)
            w = 64
            while w >= 8:
                nc.vector.tensor_tensor(
                    out=pB3[:, :, :w], in0=pB3[:, :, :w],
                    in1=pB3[:, :, w:2 * w], op=ADD)
                w //= 2
            with nc.allow_low_precision(reason="one-hot, exact"):
                nc.vector.tensor_reduce(
                    out=dB_sb[:], in_=pB3[:, :, :8], axis=AXX, op=ADD)
            gsB_sb = cst.tile([TILE_P, NB_BLOCKS * C], f32)
            gsB3 = gsB_sb[:].rearrange("p (j e) -> p j e", e=C)
            nc.vector.tensor_tensor(
                out=gsB3, in0=gB3,
                in1=dB_sb[:].to_broadcast([TILE_P, NB_BLOCKS, C]), op=MUL)

            s0 = 0
            prev = None
            for o, cap in enumerate(B_CAPS):
                e0 = s0 + cap
                sc = nc.gpsimd.dma_scatter_add(
                    out3_t, gsB3[:, s0:e0, :],
                    sidB_sb[:, WIN * s0:WIN * e0],
                    128 * cap, 128 * cap, C, single_packet=False,
                    queue_num=2)
                if prev is not None:
                    add_dep_helper(sc.ins, prev.ins, reason="B scatter chain")
                prev = sc
                s0 = e0
    nc.compile()
    return nc


def _get_program():
    if "nc" not in _CACHED:
        _CACHED["nc"] = _build_program()
    return _CACHED["nc"]


# ---------------------------------------------------------------- entry
def kernel(depth, feat, ranks_depth, ranks_feat, ranks_bev,
           interval_starts=None, interval_lengths=None):
    from concourse import bass_utils

    depth = np.asarray(depth, dtype=np.float32)
    feat = np.asarray(feat, dtype=np.float32)
    feat_flat = _bf16(feat.transpose(0, 1, 3, 4, 2).reshape(-1, C)) \
        .view(np.float32)
    dep_blk = _bf16(depth.reshape(N_DEP_BLK, 128)).view(np.float32)

    cores = _preprocess(ranks_depth, ranks_feat, ranks_bev)
    in_maps = []
    for k in range(NCORES):
        cd = cores[k]
        in_maps.append({
            "feat_tbl": feat_flat, "dep_tbl": dep_blk,
            "rfiA": cd["rfiA"], "rdiA": cd["rdiA"], "mskA": cd["mskA"],
            "selA": cd["selA"], "sidxA": cd["sidxA"],
            "rfiB": cd["rfiB"], "rdiB": cd["rdiB"], "selB": cd["selB"],
            "sidxB": cd["sidxB"],
        })

    nc = _get_program()
    res = bass_utils.run_bass_kernel_spmd(nc, in_maps,
                                          core_ids=list(range(NCORES)))
    _CACHED["last_results"] = res

    out_full = np.zeros((B, C, 1, 128, 128), np.float32)
    for k in range(NCORES):
        r = res.results[k]
        oc = (np.asarray(r["out"]) + np.asarray(r["out2"])
              + np.asarray(r["out3"]))[:CELLS_PER_CORE]
        b, blk = k // 4, k % 4
        out_full[b, :, 0, 32 * blk:32 * (blk + 1), :] = \
            oc.T.reshape(C, 32, 128)
    return out_full
